# revision 1
# baseline (speedup 1.0000x reference)
"""Causal attention (B=4, N=2048, D=1024) on 8 Trainium2 NeuronCores.

v2 design (vs baseline):
  * All on-chip tensors bf16 (tolerance 2e-2; measured numpy pipeline err
    ~4e-3).  Halves DMA bytes and SBUF so K^T, V and Q^T stay fully
    SBUF-resident -- no DRAM spill roundtrips.
  * Scores computed TRANSPOSED (S^T[k,q] per key tile): the exp'd P^T is
    directly the stationary operand of the AV matmul, eliminating all PE
    transposes and the exp->transpose->copy->AV serial chain.  Row sums
    come from a 1-wide matmul against a ones vector that reuses the AV
    stationary (P^T) already loaded in the array.
  * Core 2b+s handles batch b; s=0 takes query tiles {0,2,4,6, 9,11,13,15},
    s=1 takes {1,3,5,7, 8,10,12,14} -- both sum to 68 causal key-tile pairs.
    The program is SPMD-uniform with key limits (2,4,..,16); the per-core
    diagonal/full masks are passed as input data ([128,512] = group1 pair +
    group2 pair of 128-col blocks).
  * Slot groups of 4 give 512-wide moving operands for S^T; widths taper
    (512/384/256/128) following the causal staircase.
  * x double-buffered across the two key-tile phases; weights loaded once.
"""
import sys

sys.path.insert(0, "/opt/trn_rl_repo")

from contextlib import ExitStack

import numpy as np
import ml_dtypes

import concourse.bass as bass
import concourse.mybir as mybir
import concourse.tile as tile
from concourse import bacc
from concourse.bass_utils import run_bass_kernel_spmd

B, N, D = 4, 2048, 1024
N_CORES = 8
N_SLOTS = 8
N_KTILES = 16
SCALE = 1.0 / 32.0   # 1/sqrt(D)
NEG = -1.0e9

F32 = mybir.dt.float32
BF16 = mybir.dt.bfloat16
BF = ml_dtypes.bfloat16

# query-tile sets per parity slot s (ascending); both have sum(g+1) == 68
QSETS = [
    [0, 2, 4, 6, 9, 11, 13, 15],
    [1, 3, 5, 7, 8, 10, 12, 14],
]
# uniform program limits per slot (key tiles 0..L-1 computed)
LIMITS = [2, 4, 6, 8, 10, 12, 14, 16]

_NC_CACHE = {}
TRACE = False
LAST_EXEC_NS = None


def _build_nc():
    nc = bacc.Bacc(None, target_bir_lowering=False, debug=False)

    # x tile layout: [tile, p=d%128, dchunk, token]
    x_t = nc.declare_dram_parameter("x_t", [N_KTILES, 128, 8, 128], BF16, isOutput=False)
    x_qt = nc.declare_dram_parameter("x_qt", [N_SLOTS, 128, 8, 128], BF16, isOutput=False)
    # weights: [p=d%128, dchunk, ecol]
    wq = nc.declare_dram_parameter("wq", [128, 8, 1024], BF16, isOutput=False)
    wk = nc.declare_dram_parameter("wk", [128, 8, 1024], BF16, isOutput=False)
    # wv is e-half-major so each half is one contiguous DMA on its own queue
    wv = nc.declare_dram_parameter("wv", [2, 128, 8, 512], BF16, isOutput=False)
    mask_in = nc.declare_dram_parameter("mask", [128, 512], F32, isOutput=False)
    out_q = nc.declare_dram_parameter("out_q", [N_SLOTS, 128, D], BF16, isOutput=True)

    with tile.TileContext(nc) as tc, ExitStack() as top:
        consts = top.enter_context(tc.tile_pool(name="consts", bufs=1))
        kt_pool = top.enter_context(tc.tile_pool(name="ktp", bufs=1))
        v_pool = top.enter_context(tc.tile_pool(name="vp", bufs=1))
        qt_pool = top.enter_context(tc.tile_pool(name="qtp", bufs=1))

        ones = consts.tile([128, 8], BF16)
        nc.vector.memset(ones, 1.0)
        mask_sb = consts.tile([128, 512], F32)

        KT = kt_pool.tile([128, 8, N], BF16)     # [p=e%128, echunk, key]
        V = v_pool.tile([128, N_KTILES, D], BF16)  # [p=key%128, ktile, e]
        QT = qt_pool.tile([128, 8, 1024], BF16)  # [p=e%128, echunk, qcol]

        with ExitStack() as ph12:
            xt_pool = ph12.enter_context(tc.tile_pool(name="xtp", bufs=2))
            qxt_pool = ph12.enter_context(tc.tile_pool(name="qxt", bufs=1))
            w_pool = ph12.enter_context(tc.tile_pool(name="wp", bufs=1))
            ps_mm = ph12.enter_context(tc.tile_pool(name="ps_mm", bufs=8, space="PSUM"))

            # spread weight DMAs across queues so they stream in parallel
            # (per-queue DMA BW is ~100-180 GB/s, well under core BW).
            # wv gates the kernel's first PE work: contiguous 1MB halves on
            # two queues land ~5us earlier than one 2MB transfer.
            # first wv half split across two queues (it gates the first PE
            # chain); second half + the rest stream behind
            wv_sb = w_pool.tile([128, 2, 8, 512], BF16, tag="wv")
            nc.scalar.dma_start(out=wv_sb[:, 0, 0:4, :], in_=wv[0][:, 0:4, :])
            nc.gpsimd.dma_start(out=wv_sb[:, 0, 4:8, :], in_=wv[0][:, 4:8, :])
            nc.sync.dma_start(out=wv_sb[:, 1], in_=wv[1][:, :, :])
            wk_sb = w_pool.tile([128, 8, 1024], BF16, tag="wk")
            wq_sb = w_pool.tile([128, 8, 1024], BF16, tag="wq")
            nc.sync.dma_start(out=wq_sb, in_=wq[:, :, :])

            QXT = qxt_pool.tile([128, 8, 8, 128], BF16, tag="qx")
            nc.sync.dma_start(
                out=QXT, in_=x_qt[:].rearrange("s p c q -> p s c q")
            )
            nc.sync.dma_start(out=mask_sb, in_=mask_in[:, :])

            def proj_phase(kh):
                xT = xt_pool.tile([128, 8, 8, 128], BF16, tag="xT", name=f"x{kh}")
                if kh == 0:
                    # per-tile DMAs: first V chain starts after one tile lands
                    for lt in range(8):
                        nc.gpsimd.dma_start(
                            out=xT[:, lt, :, :], in_=x_t[lt][:, :, :]
                        )
                    # wk rides the gpsimd queue behind the kh0 x tiles;
                    # K proj needs it only after the 8 V chains (~25us in)
                    nc.gpsimd.dma_start(out=wk_sb, in_=wk[:, :, :])
                else:
                    # bulk prefetch on the scalar queue (free after wv)
                    nc.scalar.dma_start(
                        out=xT, in_=x_t[8:16].rearrange("t p c q -> p t c q")
                    )
                # V projection: stationary x chunk shared across both e-halves.
                # The first two kh0 tiles run e-halves sequentially so the
                # first chain depends only on the first wv half-DMA.
                for lt in range(8):
                    t = kh * 8 + lt
                    vps = [ps_mm.tile([128, 512], F32, tag="mm", name=f"v{t}_{eh}")
                           for eh in range(2)]
                    if kh == 0 and lt < 2:
                        for eh in range(2):
                            for c in range(8):
                                nc.tensor.matmul(
                                    vps[eh], xT[:, lt, c, :], wv_sb[:, eh, c, :],
                                    start=(c == 0), stop=(c == 7),
                                )
                    else:
                        for c in range(8):
                            for eh in range(2):
                                nc.tensor.matmul(
                                    vps[eh], xT[:, lt, c, :], wv_sb[:, eh, c, :],
                                    start=(c == 0), stop=(c == 7),
                                )
                    for eh in range(2):
                        nc.vector.tensor_copy(V[:, t, eh * 512:(eh + 1) * 512], vps[eh])
                # K^T projection: stationary W chunk shared across both key groups
                for e in range(8):
                    kps = [ps_mm.tile([128, 512], F32, tag="mm", name=f"k{kh}_{e}_{g}")
                           for g in range(2)]
                    for c in range(8):
                        for kg in range(2):
                            nc.tensor.matmul(
                                kps[kg], wk_sb[:, c, e * 128:(e + 1) * 128],
                                xT[:, kg * 4:(kg + 1) * 4, c, :],
                                start=(c == 0), stop=(c == 7),
                            )
                    for kg in range(2):
                        key0 = (kh * 8 + kg * 4) * 128
                        nc.vector.tensor_copy(KT[:, e, key0:key0 + 512], kps[kg])

            def proj_queries():
                # both slot groups; stationary W chunk shared across groups
                for e in range(8):
                    qps = [ps_mm.tile([128, 512], F32, tag="mm", name=f"q{e}_{g}")
                           for g in range(2)]
                    for c in range(8):
                        for g in range(2):
                            nc.tensor.matmul(
                                qps[g], wq_sb[:, c, e * 128:(e + 1) * 128],
                                QXT[:, g * 4:(g + 1) * 4, c, :],
                                start=(c == 0), stop=(c == 7),
                            )
                    for g in range(2):
                        nc.vector.tensor_copy(QT[:, e, g * 512:(g + 1) * 512], qps[g])

            proj_phase(0)
            proj_queries()
            proj_phase(1)

        # ---- attention: S^T per key tile, then AV with P^T stationary ----
        with ExitStack() as ph3:
            pt_pool = ph3.enter_context(tc.tile_pool(name="ptp", bufs=1))
            ps_st = ph3.enter_context(tc.tile_pool(name="ps_st", bufs=3, space="PSUM"))
            ps_o = ph3.enter_context(tc.tile_pool(name="ps_o", bufs=2, space="PSUM"))
            ps_rs = ph3.enter_context(tc.tile_pool(name="ps_rs", bufs=1, space="PSUM"))
            sc_pool = ph3.enter_context(tc.tile_pool(name="scp", bufs=2))
            outp = ph3.enter_context(tc.tile_pool(name="outp", bufs=2))

            PTs = [
                pt_pool.tile([128, 8, 512], BF16, tag="pt1", name="PT1"),
                pt_pool.tile([128, 16, 512], BF16, tag="pt2", name="PT2"),
            ]

            def st_fused():
                # one pass over key tiles; each KT stationary chunk serves
                # BOTH slot groups' S^T matmuls (halves LDWEIGHTS for kt<8)
                for kt in range(16):
                    work = []   # (group, sps, w, col0, f)
                    for g in ((1, 0) if kt < 8 else (1,)):
                        Ls = LIMITS[g * 4:(g + 1) * 4]
                        f = sum(1 for L in Ls if L <= kt)
                        w = (4 - f) * 128
                        col0 = f * 128
                        sps = ps_st.tile([128, 512], F32, tag="st",
                                         name=f"s{g}_{kt}")
                        work.append((g, sps, w, col0, f))
                    for c in range(8):
                        for g, sps, w, col0, f in work:
                            nc.tensor.matmul(
                                sps[:, 0:w],
                                KT[:, c, kt * 128:(kt + 1) * 128],
                                QT[:, c, g * 512 + col0: g * 512 + col0 + w],
                                start=(c == 0), stop=(c == 7),
                            )
                    for g, sps, w, col0, f in work:
                        Ls = LIMITS[g * 4:(g + 1) * 4]
                        if kt == Ls[f] - 2:
                            nc.vector.tensor_add(
                                sps[:, 0:128], sps[:, 0:128],
                                mask_sb[:, g * 256: g * 256 + 128],
                            )
                        elif kt == Ls[f] - 1:
                            nc.vector.tensor_add(
                                sps[:, 0:128], sps[:, 0:128],
                                mask_sb[:, g * 256 + 128: g * 256 + 256],
                            )
                        nc.scalar.activation(
                            PTs[g][:, kt, col0:col0 + w], sps[:, 0:w],
                            mybir.ActivationFunctionType.Exp,
                            bias=0.0, scale=SCALE,
                        )

            def av_slot(g, j):
                PT = PTs[g]
                slot = g * 4 + j
                L = LIMITS[slot]
                col = j * 128
                O_ps = ps_o.tile([128, D], F32, tag="O", name=f"O{slot}")
                rs_ps = ps_rs.tile([128, 1], F32, tag="rs", name=f"r{slot}")
                for kt in range(L):
                    pt_blk = PT[:, kt, col:col + 128]
                    for h in range(2):
                        nc.tensor.matmul(
                            O_ps[:, h * 512:(h + 1) * 512], pt_blk,
                            V[:, kt, h * 512:(h + 1) * 512],
                            start=(kt == 0), stop=(kt == L - 1),
                        )
                    nc.tensor.matmul(
                        rs_ps, pt_blk, ones[:, 0:1],
                        start=(kt == 0), stop=(kt == L - 1),
                    )
                stats = sc_pool.tile([128, 8], F32, tag="stats", name=f"st{slot}")
                recip = stats[:, 0:1]
                nc.vector.reciprocal(recip, rs_ps)
                out_sb = outp.tile([128, D], BF16, tag="osb", name=f"ou{slot}")
                nc.vector.tensor_scalar_mul(out_sb, O_ps, recip)
                # alternate output queues so the final drain is parallel
                eng = nc.sync if slot % 2 == 0 else nc.gpsimd
                eng.dma_start(out=out_q[slot][:, :], in_=out_sb)

            # descending L within each group: the big slots' outputs DMA out
            # early, shrinking the end-of-kernel drain
            # interleave big(g2)/small(g1) slots in descending L: each small
            # slot's recip/scale/DMA epilogue hides under the next big slot's
            # matmul chain, and the last emitted slot (L=2) has the shortest
            # end-of-kernel chain
            st_fused()
            for j in (3, 2, 1, 0):
                av_slot(1, j)
                av_slot(0, j)

    nc.compile()
    return nc


def _masks():
    k = np.arange(128)[:, None]
    q = np.arange(128)[None, :]
    tril_t = np.where(k <= q, 0.0, NEG).astype(np.float32)  # S^T diag block
    fullneg = np.full((128, 128), NEG, np.float32)
    zeros = np.zeros((128, 128), np.float32)
    m_s0 = np.concatenate([tril_t, fullneg, zeros, tril_t], axis=1)
    m_s1 = np.concatenate([zeros, tril_t, tril_t, fullneg], axis=1)
    return m_s0, m_s1


def kernel(x, Wq, Wk, Wv):
    global LAST_EXEC_NS
    x = np.asarray(x, dtype=np.float32)
    Wq = np.asarray(Wq, dtype=np.float32)
    Wk = np.asarray(Wk, dtype=np.float32)
    Wv = np.asarray(Wv, dtype=np.float32)

    if "nc" not in _NC_CACHE:
        _NC_CACHE["nc"] = _build_nc()
    nc = _NC_CACHE["nc"]

    # host pre-transpose: x[b] (N, D) -> (tile, p=d%128, dchunk, token), bf16
    xt_all = np.ascontiguousarray(
        x.reshape(B, N_KTILES, 128, 8, 128).transpose(0, 1, 4, 3, 2).astype(BF)
    )  # [B, tile, p, c, q]

    # weights -> [p=d%128, dchunk, ecol], bf16
    wq_r = np.ascontiguousarray(Wq.reshape(8, 128, 1024).transpose(1, 0, 2).astype(BF))
    wk_r = np.ascontiguousarray(Wk.reshape(8, 128, 1024).transpose(1, 0, 2).astype(BF))
    # wv: [eh, p=d%128, dchunk, ecol]
    wv_r = np.ascontiguousarray(
        Wv.reshape(8, 128, 2, 512).transpose(2, 1, 0, 3).astype(BF))

    m_s0, m_s1 = _masks()
    in_maps = []
    for c in range(N_CORES):
        b, s = divmod(c, 2)
        in_maps.append({
            "x_t": xt_all[b],
            "x_qt": np.ascontiguousarray(xt_all[b, QSETS[s]]),
            "wq": wq_r, "wk": wk_r, "wv": wv_r,
            "mask": m_s1 if s else m_s0,
        })

    res = run_bass_kernel_spmd(nc, in_maps, list(range(N_CORES)), trace=TRACE)
    LAST_EXEC_NS = res.exec_time_ns

    out = np.empty((B, N, D), dtype=np.float32)
    for c in range(N_CORES):
        b, s = divmod(c, 2)
        oq = np.asarray(res.results[c]["out_q"], dtype=np.float32)
        for j, g in enumerate(QSETS[s]):
            out[b, g * 128:(g + 1) * 128, :] = oq[j]
    return out



# revision 2
# speedup vs baseline: 1.0777x; 1.0777x over previous
"""Causal attention (B=4, N=2048, D=1024) on 8 Trainium2 NeuronCores.

v3 design (vs v2 all-bf16 baseline, 235us):
  * fp8(e4m3) DoubleRow matmuls for the V projection, S^T and AV --
    0.565 cyc/col vs 1.0 bf16.  Q/K projections stay bf16 (fp8 q/k
    injects ~2% score noise that softmax amplifies past tolerance).
    Scale factors folded into host-side weights so no on-chip rescale:
      wq,wk = 4*W (bf16)   -> psum = 4q, cast straight to fp8 (|4q|<~12)
      wv8   = e4m3(32*Wv)  -> psum = 32v, cast to fp8 (|32v|<~95<240)
      exp scale = (1/sqrt(D))/16;  rowsum ones = 32.0 so the 32 cancels
      in out = O_psum * (1/rowsum).
  * Early-row fixup: rows where softmax concentrates on few keys expose
    raw fp8 V error (~5%).  Each core computes its L=2 slot (q-tile 0
    for s=0 cores, q-tile 1 for s=1) in bf16: bf16 K^T/Q^T copies for
    kt0/1, bf16 V for kt0/1 (extra bf16 Wv matmul), bf16 P and AV.
    Collectively rows 0..255 of every batch get the bf16 path;
    numpy-sim of this exact mix: max rel err 4.4e-3 (tolerance 2e-2).
  * DMA priority order: first V-proj chain needs only wv8[c0:2]+x8[t0]
    (0.4 MB) -- those go first on the sync ring; K-path (wk, x bf16)
    next on gpsimd; Q-path + fixup weights last on scalar.  PE phase
    order V0 V1 K0a K0b K1 Q Vb matches the DMA arrival order.
  * Core 2b+s handles batch b; s=0 takes query tiles {0,2,4,6, 9,11,13,15},
    s=1 takes {1,3,5,7, 8,10,12,14} -- both sum to 68 causal key-tile
    pairs.  SPMD-uniform program; per-parity masks passed as data.
"""
import sys

sys.path.insert(0, "/opt/trn_rl_repo")

from contextlib import ExitStack

import numpy as np
import ml_dtypes

import concourse.bass as bass
import concourse.mybir as mybir
import concourse.tile as tile
from concourse import bacc
from concourse.bass_utils import run_bass_kernel_spmd

B, N, D = 4, 2048, 1024
N_CORES = 8
N_SLOTS = 8
N_KTILES = 16
SCALE = 1.0 / 32.0   # 1/sqrt(D)
QK_PREMUL = 4.0      # folded into wq/wk on host
V_PREMUL = 32.0      # folded into wv on host
EXP_SCALE = SCALE / (QK_PREMUL * QK_PREMUL)
NEG = -1.0e9

F32 = mybir.dt.float32
BF16 = mybir.dt.bfloat16
F8 = mybir.dt.float8e4
DR = mybir.MatmulPerfMode.DoubleRow
BF = ml_dtypes.bfloat16
F8NP = ml_dtypes.float8_e4m3

# query-tile sets per parity slot s (ascending); both have sum(g+1) == 68
QSETS = [
    [0, 2, 4, 6, 9, 11, 13, 15],
    [1, 3, 5, 7, 8, 10, 12, 14],
]
# uniform program limits per slot (key tiles 0..L-1 computed)
LIMITS = [2, 4, 6, 8, 10, 12, 14, 16]

_NC_CACHE = {}
TRACE = False
LAST_EXEC_NS = None


def _build_nc():
    nc = bacc.Bacc(None, target_bir_lowering=False, debug=False)

    # x tile layouts: [tile, p=d%128, dchunk, token]
    x_t = nc.declare_dram_parameter("x_t", [N_KTILES, 128, 8, 128], BF16, isOutput=False)
    x_t8 = nc.declare_dram_parameter("x_t8", [N_KTILES, 128, 8, 128], F8, isOutput=False)
    x_qt = nc.declare_dram_parameter("x_qt", [N_SLOTS, 128, 8, 128], BF16, isOutput=False)
    # weights: [p=d%128, dchunk, ecol]
    wq = nc.declare_dram_parameter("wq", [128, 8, 1024], BF16, isOutput=False)
    wk = nc.declare_dram_parameter("wk", [128, 8, 1024], BF16, isOutput=False)
    wv8 = nc.declare_dram_parameter("wv8", [128, 8, 1024], F8, isOutput=False)
    wvb = nc.declare_dram_parameter("wvb", [128, 8, 1024], BF16, isOutput=False)
    mask_in = nc.declare_dram_parameter("mask", [128, 512], F32, isOutput=False)
    out_q = nc.declare_dram_parameter("out_q", [N_SLOTS, 128, D], BF16, isOutput=True)

    with tile.TileContext(nc) as tc, ExitStack() as top:
        consts = top.enter_context(tc.tile_pool(name="consts", bufs=1))
        kt_pool = top.enter_context(tc.tile_pool(name="ktp", bufs=1))
        v_pool = top.enter_context(tc.tile_pool(name="vp", bufs=1))
        qt_pool = top.enter_context(tc.tile_pool(name="qtp", bufs=1))

        ones8 = consts.tile([128, 2, 16], F8)
        nc.vector.memset(ones8, V_PREMUL)
        onesb = consts.tile([128, 8], BF16)
        nc.vector.memset(onesb, V_PREMUL)
        mask_sb = consts.tile([128, 512], F32)

        KT8 = kt_pool.tile([128, 8, N], F8)        # [p=e%128, echunk, key]
        KTb = kt_pool.tile([128, 8, 256], BF16)    # bf16 keys 0..255 (fixup)
        QT8 = qt_pool.tile([128, 8, 1024], F8)     # [p=e%128, echunk, qcol]
        QTb = qt_pool.tile([128, 8, 128], BF16)    # bf16 fixup q-tile (g0 col 0)
        V8 = v_pool.tile([128, N_KTILES, D], F8)   # [p=key%128, ktile, e]
        Vb = v_pool.tile([128, 2, D], BF16)        # bf16 V kt0/1 (fixup)

        with ExitStack() as ph12:
            x8_pool = ph12.enter_context(tc.tile_pool(name="x8p", bufs=2))
            xt_pool = ph12.enter_context(tc.tile_pool(name="xtp", bufs=2))
            qxt_pool = ph12.enter_context(tc.tile_pool(name="qxt", bufs=1))
            w_pool = ph12.enter_context(tc.tile_pool(name="wp", bufs=1))
            ps_mm = ph12.enter_context(tc.tile_pool(name="ps_mm", bufs=8, space="PSUM"))

            # ---- DMA issue, priority order ----
            # critical stream (sync ring): wv8 head chunks + x8 tiles --
            # first V-proj chain starts after ~0.4 MB lands.
            wv8_sb = w_pool.tile([128, 8, 1024], F8, tag="wv8")
            nc.sync.dma_start(out=wv8_sb[:, 0:2, :], in_=wv8[:, 0:2, :])
            x8a = x8_pool.tile([128, 8, 8, 128], F8, tag="x8", name="x8a")
            x8b = x8_pool.tile([128, 8, 8, 128], F8, tag="x8", name="x8b")
            for t in range(8):
                nc.sync.dma_start(out=x8a[:, t], in_=x_t8[t][:, :, :])
            nc.sync.dma_start(out=wv8_sb[:, 2:8, :], in_=wv8[:, 2:8, :])
            for t in range(8):
                nc.sync.dma_start(out=x8b[:, t], in_=x_t8[8 + t][:, :, :])
            # K-path stream (gpsimd ring): wk then x bf16 tiles
            wk_sb = w_pool.tile([128, 8, 1024], BF16, tag="wk")
            nc.gpsimd.dma_start(out=wk_sb, in_=wk[:, :, :])
            xT0 = xt_pool.tile([128, 8, 8, 128], BF16, tag="xT", name="xT0")
            xT1 = xt_pool.tile([128, 8, 8, 128], BF16, tag="xT", name="xT1")
            nc.gpsimd.dma_start(
                out=xT0[:, 0:4], in_=x_t[0:4].rearrange("t p c q -> p t c q"))
            nc.gpsimd.dma_start(
                out=xT0[:, 4:8], in_=x_t[4:8].rearrange("t p c q -> p t c q"))
            nc.gpsimd.dma_start(
                out=xT1, in_=x_t[8:16].rearrange("t p c q -> p t c q"))
            # Q-path + fixup stream (scalar ring): needed latest
            QXT = qxt_pool.tile([128, 8, 8, 128], BF16, tag="qx")
            nc.scalar.dma_start(
                out=QXT, in_=x_qt[:].rearrange("s p c q -> p s c q"))
            wq_sb = w_pool.tile([128, 8, 1024], BF16, tag="wq")
            nc.scalar.dma_start(out=wq_sb, in_=wq[:, :, :])
            wvb_sb = w_pool.tile([128, 8, 1024], BF16, tag="wvb")
            nc.scalar.dma_start(out=wvb_sb, in_=wvb[:, :, :])
            nc.scalar.dma_start(out=mask_sb, in_=mask_in[:, :])

            def v_phase(x8t, t0):
                # fp8 DoubleRow: stationary x chunk-pair shared by both e-halves
                for lt in range(8):
                    vps = [ps_mm.tile([128, 512], F32, tag="mm", name=f"v{t0+lt}_{eh}")
                           for eh in range(2)]
                    for c2 in range(4):
                        for eh in range(2):
                            nc.tensor.matmul(
                                vps[eh],
                                x8t[:, lt, 2 * c2:2 * c2 + 2, :],
                                wv8_sb[:, 2 * c2:2 * c2 + 2, eh * 512:(eh + 1) * 512],
                                start=(c2 == 0), stop=(c2 == 3),
                                perf_mode=DR,
                            )
                    for eh in range(2):
                        nc.vector.tensor_copy(
                            V8[:, t0 + lt, eh * 512:(eh + 1) * 512], vps[eh])

            def k_pass(xTt, kh, tg):
                # bf16 K^T projection for one 4-tile group
                for e in range(8):
                    kps = ps_mm.tile([128, 512], F32, tag="mm", name=f"k{kh}{tg}_{e}")
                    for c in range(8):
                        nc.tensor.matmul(
                            kps, wk_sb[:, c, e * 128:(e + 1) * 128],
                            xTt[:, tg * 4:(tg + 1) * 4, c, :],
                            start=(c == 0), stop=(c == 7),
                        )
                    key0 = (kh * 8 + tg * 4) * 128
                    nc.vector.tensor_copy(KT8[:, e, key0:key0 + 512], kps)
                    if kh == 0 and tg == 0:
                        nc.vector.tensor_copy(KTb[:, e, :], kps[:, 0:256])

            def k_pass_shared(xTt, kh):
                # bf16 K^T, stationary wk chunk shared across both 4-tile groups
                for e in range(8):
                    kps = [ps_mm.tile([128, 512], F32, tag="mm", name=f"k{kh}_{e}_{g}")
                           for g in range(2)]
                    for c in range(8):
                        for kg in range(2):
                            nc.tensor.matmul(
                                kps[kg], wk_sb[:, c, e * 128:(e + 1) * 128],
                                xTt[:, kg * 4:(kg + 1) * 4, c, :],
                                start=(c == 0), stop=(c == 7),
                            )
                    for kg in range(2):
                        key0 = (kh * 8 + kg * 4) * 128
                        nc.vector.tensor_copy(KT8[:, e, key0:key0 + 512], kps[kg])

            def proj_queries():
                # both slot groups; stationary W chunk shared across groups
                for e in range(8):
                    qps = [ps_mm.tile([128, 512], F32, tag="mm", name=f"q{e}_{g}")
                           for g in range(2)]
                    for c in range(8):
                        for g in range(2):
                            nc.tensor.matmul(
                                qps[g], wq_sb[:, c, e * 128:(e + 1) * 128],
                                QXT[:, g * 4:(g + 1) * 4, c, :],
                                start=(c == 0), stop=(c == 7),
                            )
                    for g in range(2):
                        nc.vector.tensor_copy(QT8[:, e, g * 512:(g + 1) * 512], qps[g])
                    nc.vector.tensor_copy(QTb[:, e, :], qps[0][:, 0:128])

            def vb_fix():
                # bf16 V for key tiles 0,1 (fixup slot); stationary x chunk
                # shared across e-halves
                for t in range(2):
                    vbp = [ps_mm.tile([128, 512], F32, tag="mm", name=f"vb{t}_{eh}")
                           for eh in range(2)]
                    for c in range(8):
                        for eh in range(2):
                            nc.tensor.matmul(
                                vbp[eh], xT0[:, t, c, :],
                                wvb_sb[:, c, eh * 512:(eh + 1) * 512],
                                start=(c == 0), stop=(c == 7),
                            )
                    for eh in range(2):
                        nc.vector.tensor_copy(
                            Vb[:, t, eh * 512:(eh + 1) * 512], vbp[eh])

            v_phase(x8a, 0)
            v_phase(x8b, 8)
            k_pass(xT0, 0, 0)
            k_pass(xT0, 0, 1)
            k_pass_shared(xT1, 1)
            proj_queries()
            vb_fix()

        # ---- attention: S^T per key tile, then AV with P^T stationary ----
        with ExitStack() as ph3:
            pt_pool = ph3.enter_context(tc.tile_pool(name="ptp", bufs=1))
            ps_st = ph3.enter_context(tc.tile_pool(name="ps_st", bufs=3, space="PSUM"))
            ps_o = ph3.enter_context(tc.tile_pool(name="ps_o", bufs=2, space="PSUM"))
            ps_rs = ph3.enter_context(tc.tile_pool(name="ps_rs", bufs=1, space="PSUM"))
            sc_pool = ph3.enter_context(tc.tile_pool(name="scp", bufs=2))
            outp = ph3.enter_context(tc.tile_pool(name="outp", bufs=2))

            PTs = [
                pt_pool.tile([128, 8, 512], F8, tag="pt1", name="PT1"),
                pt_pool.tile([128, 16, 512], F8, tag="pt2", name="PT2"),
            ]
            Pb = pt_pool.tile([128, 2, 128], BF16, tag="pb", name="Pb")

            def st_fused():
                # one pass over key tiles; each KT stationary chunk-pair
                # serves BOTH slot groups' S^T matmuls (kt<8)
                for kt in range(16):
                    work = []   # (group, sps, w, col0, f)
                    for g in ((1, 0) if kt < 8 else (1,)):
                        Ls = LIMITS[g * 4:(g + 1) * 4]
                        f = sum(1 for L in Ls if L <= kt)
                        w = (4 - f) * 128
                        col0 = f * 128
                        sps = ps_st.tile([128, 512], F32, tag="st",
                                         name=f"s{g}_{kt}")
                        work.append((g, sps, w, col0, f))
                    for c2 in range(4):
                        for g, sps, w, col0, f in work:
                            nc.tensor.matmul(
                                sps[:, 0:w],
                                KT8[:, 2 * c2:2 * c2 + 2, kt * 128:(kt + 1) * 128],
                                QT8[:, 2 * c2:2 * c2 + 2,
                                    g * 512 + col0: g * 512 + col0 + w],
                                start=(c2 == 0), stop=(c2 == 3),
                                perf_mode=DR,
                            )
                    for g, sps, w, col0, f in work:
                        Ls = LIMITS[g * 4:(g + 1) * 4]
                        if kt == Ls[f] - 2:
                            nc.vector.tensor_add(
                                sps[:, 0:128], sps[:, 0:128],
                                mask_sb[:, g * 256: g * 256 + 128],
                            )
                        elif kt == Ls[f] - 1:
                            nc.vector.tensor_add(
                                sps[:, 0:128], sps[:, 0:128],
                                mask_sb[:, g * 256 + 128: g * 256 + 256],
                            )
                        nc.scalar.activation(
                            PTs[g][:, kt, col0:col0 + w], sps[:, 0:w],
                            mybir.ActivationFunctionType.Exp,
                            bias=0.0, scale=EXP_SCALE,
                        )

            def st_fix():
                # bf16 S^T for the fixup slot (g0 col0, key tiles 0,1)
                for kt in range(2):
                    spb = ps_st.tile([128, 512], F32, tag="st", name=f"sf{kt}")
                    for c in range(8):
                        nc.tensor.matmul(
                            spb[:, 0:128], KTb[:, c, kt * 128:(kt + 1) * 128],
                            QTb[:, c, :],
                            start=(c == 0), stop=(c == 7),
                        )
                    nc.vector.tensor_add(
                        spb[:, 0:128], spb[:, 0:128],
                        mask_sb[:, kt * 128:(kt + 1) * 128],
                    )
                    nc.scalar.activation(
                        Pb[:, kt, :], spb[:, 0:128],
                        mybir.ActivationFunctionType.Exp,
                        bias=0.0, scale=EXP_SCALE,
                    )

            def av_epilogue(slot, O_ps, rs_ps):
                stats = sc_pool.tile([128, 8], F32, tag="stats", name=f"st{slot}")
                recip = stats[:, 0:1]
                nc.vector.reciprocal(recip, rs_ps)
                out_sb = outp.tile([128, D], BF16, tag="osb", name=f"ou{slot}")
                nc.vector.tensor_scalar_mul(out_sb, O_ps, recip)
                eng = nc.sync if slot % 2 == 0 else nc.gpsimd
                eng.dma_start(out=out_q[slot][:, :], in_=out_sb)

            def av_slot(g, j):
                # fp8 DoubleRow over key-tile pairs; rowsum reuses stationary
                PT = PTs[g]
                slot = g * 4 + j
                L = LIMITS[slot]
                col = j * 128
                O_ps = ps_o.tile([128, D], F32, tag="O", name=f"O{slot}")
                rs_ps = ps_rs.tile([128, 1], F32, tag="rs", name=f"r{slot}")
                L2 = L // 2
                for t2 in range(L2):
                    pt_blk = PT[:, 2 * t2:2 * t2 + 2, col:col + 128]
                    for h in range(2):
                        nc.tensor.matmul(
                            O_ps[:, h * 512:(h + 1) * 512], pt_blk,
                            V8[:, 2 * t2:2 * t2 + 2, h * 512:(h + 1) * 512],
                            start=(t2 == 0), stop=(t2 == L2 - 1),
                            perf_mode=DR,
                        )
                    nc.tensor.matmul(
                        rs_ps, pt_blk, ones8[:, :, 0:1],
                        start=(t2 == 0), stop=(t2 == L2 - 1),
                        perf_mode=DR,
                    )
                av_epilogue(slot, O_ps, rs_ps)

            def av_fix():
                # bf16 AV for the fixup slot (slot 0, L=2)
                O_ps = ps_o.tile([128, D], F32, tag="O", name="Ofix")
                rs_ps = ps_rs.tile([128, 1], F32, tag="rs", name="rfix")
                for kt in range(2):
                    pb_blk = Pb[:, kt, :]
                    for h in range(2):
                        nc.tensor.matmul(
                            O_ps[:, h * 512:(h + 1) * 512], pb_blk,
                            Vb[:, kt, h * 512:(h + 1) * 512],
                            start=(kt == 0), stop=(kt == 1),
                        )
                    nc.tensor.matmul(
                        rs_ps, pb_blk, onesb[:, 0:1],
                        start=(kt == 0), stop=(kt == 1),
                    )
                av_epilogue(0, O_ps, rs_ps)

            # interleave big(g1)/small(g0) slots in descending L; fixup slot
            # (L=2, bf16) last so the end-of-kernel chain is shortest
            st_fused()
            st_fix()
            for j in (3, 2, 1):
                av_slot(1, j)
                av_slot(0, j)
            av_slot(1, 0)
            av_fix()

    nc.compile()
    return nc


def _masks():
    k = np.arange(128)[:, None]
    q = np.arange(128)[None, :]
    tril_t = np.where(k <= q, 0.0, NEG).astype(np.float32)  # S^T diag block
    fullneg = np.full((128, 128), NEG, np.float32)
    zeros = np.zeros((128, 128), np.float32)
    m_s0 = np.concatenate([tril_t, fullneg, zeros, tril_t], axis=1)
    m_s1 = np.concatenate([zeros, tril_t, tril_t, fullneg], axis=1)
    return m_s0, m_s1


def kernel(x, Wq, Wk, Wv):
    global LAST_EXEC_NS
    x = np.asarray(x, dtype=np.float32)
    Wq = np.asarray(Wq, dtype=np.float32)
    Wk = np.asarray(Wk, dtype=np.float32)
    Wv = np.asarray(Wv, dtype=np.float32)

    if "nc" not in _NC_CACHE:
        _NC_CACHE["nc"] = _build_nc()
    nc = _NC_CACHE["nc"]

    # host pre-transpose: x[b] (N, D) -> (tile, p=d%128, dchunk, token)
    xt_f32 = np.ascontiguousarray(
        x.reshape(B, N_KTILES, 128, 8, 128).transpose(0, 1, 4, 3, 2)
    )  # [B, tile, p, c, q] f32
    xt_all = xt_f32.astype(BF)
    x8_all = xt_f32.astype(F8NP)

    # weights -> [p=d%128, dchunk, ecol]; premuls folded in
    wq_r = np.ascontiguousarray(
        (QK_PREMUL * Wq).reshape(8, 128, 1024).transpose(1, 0, 2).astype(BF))
    wk_r = np.ascontiguousarray(
        (QK_PREMUL * Wk).reshape(8, 128, 1024).transpose(1, 0, 2).astype(BF))
    wv_scaled = np.ascontiguousarray(
        (V_PREMUL * Wv).reshape(8, 128, 1024).transpose(1, 0, 2))
    wv8_r = wv_scaled.astype(F8NP)
    wvb_r = wv_scaled.astype(BF)

    m_s0, m_s1 = _masks()
    in_maps = []
    for c in range(N_CORES):
        b, s = divmod(c, 2)
        in_maps.append({
            "x_t": xt_all[b],
            "x_t8": x8_all[b],
            "x_qt": np.ascontiguousarray(xt_all[b, QSETS[s]]),
            "wq": wq_r, "wk": wk_r, "wv8": wv8_r, "wvb": wvb_r,
            "mask": m_s1 if s else m_s0,
        })

    res = run_bass_kernel_spmd(nc, in_maps, list(range(N_CORES)), trace=TRACE)
    LAST_EXEC_NS = res.exec_time_ns

    out = np.empty((B, N, D), dtype=np.float32)
    for c in range(N_CORES):
        b, s = divmod(c, 2)
        oq = np.asarray(res.results[c]["out_q"], dtype=np.float32)
        for j, g in enumerate(QSETS[s]):
            out[b, g * 128:(g + 1) * 128, :] = oq[j]
    return out


# revision 4
# speedup vs baseline: 1.2856x; 1.1929x over previous
"""Causal attention (B=4, N=2048, D=1024) on 8 Trainium2 NeuronCores.

v3 design (vs v2 all-bf16 baseline, 235us):
  * fp8(e4m3) DoubleRow matmuls for the V projection, S^T and AV --
    0.565 cyc/col vs 1.0 bf16.  Q/K projections stay bf16 (fp8 q/k
    injects ~2% score noise that softmax amplifies past tolerance).
    Scale factors folded into host-side weights so no on-chip rescale:
      wq,wk = 4*W (bf16)   -> psum = 4q, cast straight to fp8 (|4q|<~12)
      wv8   = e4m3(32*Wv)  -> psum = 32v, cast to fp8 (|32v|<~95<240)
      exp scale = (1/sqrt(D))/16;  rowsum ones = 32.0 so the 32 cancels
      in out = O_psum * (1/rowsum).
  * Early-row fixup: rows where softmax concentrates on few keys expose
    raw fp8 V error (~5%).  Each core computes its L=2 slot (q-tile 0
    for s=0 cores, q-tile 1 for s=1) in bf16: bf16 K^T/Q^T copies for
    kt0/1, bf16 V for kt0/1 (extra bf16 Wv matmul), bf16 P and AV.
    Collectively rows 0..255 of every batch get the bf16 path;
    numpy-sim of this exact mix: max rel err 4.4e-3 (tolerance 2e-2).
  * DMA priority order: first V-proj chain needs only wv8[c0:2]+x8[t0]
    (0.4 MB) -- those go first on the sync ring; K-path (wk, x bf16)
    next on gpsimd; Q-path + fixup weights last on scalar.  PE phase
    order V0 V1 K0a K0b K1 Q Vb matches the DMA arrival order.
  * Core 2b+s handles batch b; s=0 takes query tiles {0,2,4,6, 9,11,13,15},
    s=1 takes {1,3,5,7, 8,10,12,14} -- both sum to 68 causal key-tile
    pairs.  SPMD-uniform program; per-parity masks passed as data.
"""
import sys

sys.path.insert(0, "/opt/trn_rl_repo")

from contextlib import ExitStack

import numpy as np
import ml_dtypes

import concourse.bass as bass
import concourse.mybir as mybir
import concourse.tile as tile
from concourse import bacc
from concourse.bass_utils import run_bass_kernel_spmd

B, N, D = 4, 2048, 1024
N_CORES = 8
N_SLOTS = 8
N_KTILES = 16
SCALE = 1.0 / 32.0   # 1/sqrt(D)
QK_PREMUL = 4.0      # folded into wq/wk on host
V_PREMUL = 32.0      # folded into wv on host
EXP_SCALE = SCALE / (QK_PREMUL * QK_PREMUL)
NEG = -1.0e9

F32 = mybir.dt.float32
BF16 = mybir.dt.bfloat16
F8 = mybir.dt.float8e4
DR = mybir.MatmulPerfMode.DoubleRow
BF = ml_dtypes.bfloat16
F8NP = ml_dtypes.float8_e4m3

# query-tile sets per parity slot s (ascending); both have sum(g+1) == 68
QSETS = [
    [0, 2, 4, 6, 9, 11, 13, 15],
    [1, 3, 5, 7, 8, 10, 12, 14],
]
# uniform program limits per slot (key tiles 0..L-1 computed)
LIMITS = [2, 4, 6, 8, 10, 12, 14, 16]

_NC_CACHE = {}
TRACE = False
LAST_EXEC_NS = None


def _build_nc():
    nc = bacc.Bacc(None, target_bir_lowering=False, debug=False)

    # x tile layouts: [tile, p=d%128, dchunk, token]
    x_t = nc.declare_dram_parameter("x_t", [N_KTILES, 128, 8, 128], BF16, isOutput=False)
    x_t8 = nc.declare_dram_parameter("x_t8", [N_KTILES, 128, 8, 128], F8, isOutput=False)
    x_qt = nc.declare_dram_parameter("x_qt", [N_SLOTS, 128, 8, 128], BF16, isOutput=False)
    # weights: [p=d%128, dchunk, ecol]
    wq = nc.declare_dram_parameter("wq", [128, 8, 1024], BF16, isOutput=False)
    wk = nc.declare_dram_parameter("wk", [128, 8, 1024], BF16, isOutput=False)
    wv8 = nc.declare_dram_parameter("wv8", [128, 8, 1024], F8, isOutput=False)
    wvb = nc.declare_dram_parameter("wvb", [128, 8, 1024], BF16, isOutput=False)
    mask_in = nc.declare_dram_parameter("mask", [128, 512], F32, isOutput=False)
    out_q = nc.declare_dram_parameter("out_q", [N_SLOTS, 128, D], BF16, isOutput=True)

    with tile.TileContext(nc) as tc, ExitStack() as top:
        consts = top.enter_context(tc.tile_pool(name="consts", bufs=1))
        kt_pool = top.enter_context(tc.tile_pool(name="ktp", bufs=1))
        v_pool = top.enter_context(tc.tile_pool(name="vp", bufs=1))
        qt_pool = top.enter_context(tc.tile_pool(name="qtp", bufs=1))

        ones8 = consts.tile([128, 2, 16], F8)
        nc.vector.memset(ones8, V_PREMUL)
        onesb = consts.tile([128, 8], BF16)
        nc.vector.memset(onesb, V_PREMUL)
        mask_sb = consts.tile([128, 512], F32)

        KT8 = kt_pool.tile([128, 8, N], F8)        # [p=e%128, echunk, key]
        KTb = kt_pool.tile([128, 8, 256], BF16)    # bf16 keys 0..255 (fixup)
        QT8 = qt_pool.tile([128, 8, 1024], F8)     # [p=e%128, echunk, qcol]
        QTb = qt_pool.tile([128, 8, 128], BF16)    # bf16 fixup q-tile (g0 col 0)
        V8 = v_pool.tile([128, N_KTILES, D], F8)   # [p=key%128, ktile, e]
        Vb = v_pool.tile([128, 2, D], BF16)        # bf16 V kt0/1 (fixup)

        with ExitStack() as ph12:
            x8_pool = ph12.enter_context(tc.tile_pool(name="x8p", bufs=2))
            xt_pool = ph12.enter_context(tc.tile_pool(name="xtp", bufs=2))
            qxt_pool = ph12.enter_context(tc.tile_pool(name="qxt", bufs=1))
            w_pool = ph12.enter_context(tc.tile_pool(name="wp", bufs=1))
            ps_mm = ph12.enter_context(tc.tile_pool(name="ps_mm", bufs=8, space="PSUM"))

            # ---- DMA issue: ONE logical queue (sync/HWDGE), strict FIFO ----
            # SDMA engines round-robin across ACTIVE queues at packet
            # granularity, so multi-ring "priority" just dilutes the critical
            # stream to 1/N of BW.  A single queue drains in issue order at
            # ~341 GB/s (1 MB+ transfers), so arrival order == issue order.
            wv8_sb = w_pool.tile([128, 8, 1024], F8, tag="wv8")
            x8a = x8_pool.tile([128, 8, 8, 128], F8, tag="x8", name="x8a")
            x8b = x8_pool.tile([128, 8, 8, 128], F8, tag="x8", name="x8b")
            wk_sb = w_pool.tile([128, 8, 1024], BF16, tag="wk")
            xT0 = xt_pool.tile([128, 8, 8, 128], BF16, tag="xT", name="xT0")
            xT1 = xt_pool.tile([128, 8, 8, 128], BF16, tag="xT", name="xT1")
            QXT = qxt_pool.tile([128, 8, 8, 128], BF16, tag="qx")
            wq_sb = w_pool.tile([128, 8, 1024], BF16, tag="wq")
            wvb_sb = w_pool.tile([128, 8, 1024], BF16, tag="wvb")

            nc.sync.dma_start(out=wv8_sb[:, 0:2, :], in_=wv8[:, 0:2, :])
            nc.sync.dma_start(
                out=x8a[:, 0:2], in_=x_t8[0:2].rearrange("t p c q -> p t c q"))
            nc.sync.dma_start(out=wv8_sb[:, 2:8, :], in_=wv8[:, 2:8, :])
            nc.sync.dma_start(
                out=x8a[:, 2:8], in_=x_t8[2:8].rearrange("t p c q -> p t c q"))
            nc.sync.dma_start(
                out=x8b, in_=x_t8[8:16].rearrange("t p c q -> p t c q"))
            nc.sync.dma_start(out=wk_sb, in_=wk[:, :, :])
            nc.sync.dma_start(
                out=xT0[:, 0:4], in_=x_t[0:4].rearrange("t p c q -> p t c q"))
            nc.sync.dma_start(
                out=xT0[:, 4:8], in_=x_t[4:8].rearrange("t p c q -> p t c q"))
            nc.sync.dma_start(
                out=xT1, in_=x_t[8:16].rearrange("t p c q -> p t c q"))
            nc.sync.dma_start(
                out=QXT, in_=x_qt[:].rearrange("s p c q -> p s c q"))
            nc.sync.dma_start(out=wq_sb, in_=wq[:, :, :])
            nc.sync.dma_start(out=wvb_sb, in_=wvb[:, :, :])
            nc.sync.dma_start(out=mask_sb, in_=mask_in[:, :])

            def v_phase(x8t, t0):
                # fp8 DoubleRow: stationary x chunk-pair shared by both e-halves
                for lt in range(8):
                    vps = [ps_mm.tile([128, 512], F32, tag="mm", name=f"v{t0+lt}_{eh}")
                           for eh in range(2)]
                    for c2 in range(4):
                        for eh in range(2):
                            nc.tensor.matmul(
                                vps[eh],
                                x8t[:, lt, 2 * c2:2 * c2 + 2, :],
                                wv8_sb[:, 2 * c2:2 * c2 + 2, eh * 512:(eh + 1) * 512],
                                start=(c2 == 0), stop=(c2 == 3),
                                perf_mode=DR,
                            )
                    for eh in range(2):
                        nc.vector.tensor_copy(
                            V8[:, t0 + lt, eh * 512:(eh + 1) * 512], vps[eh])

            def k_pass(xTt, kh, tg):
                # bf16 K^T projection for one 4-tile group
                for e in range(8):
                    kps = ps_mm.tile([128, 512], F32, tag="mm", name=f"k{kh}{tg}_{e}")
                    for c in range(8):
                        nc.tensor.matmul(
                            kps, wk_sb[:, c, e * 128:(e + 1) * 128],
                            xTt[:, tg * 4:(tg + 1) * 4, c, :],
                            start=(c == 0), stop=(c == 7),
                        )
                    key0 = (kh * 8 + tg * 4) * 128
                    nc.vector.tensor_copy(KT8[:, e, key0:key0 + 512], kps)
                    if kh == 0 and tg == 0:
                        nc.vector.tensor_copy(KTb[:, e, :], kps[:, 0:256])

            def k_pass_shared(xTt, kh):
                # bf16 K^T, stationary wk chunk shared across both 4-tile groups
                for e in range(8):
                    kps = [ps_mm.tile([128, 512], F32, tag="mm", name=f"k{kh}_{e}_{g}")
                           for g in range(2)]
                    for c in range(8):
                        for kg in range(2):
                            nc.tensor.matmul(
                                kps[kg], wk_sb[:, c, e * 128:(e + 1) * 128],
                                xTt[:, kg * 4:(kg + 1) * 4, c, :],
                                start=(c == 0), stop=(c == 7),
                            )
                    for kg in range(2):
                        key0 = (kh * 8 + kg * 4) * 128
                        nc.vector.tensor_copy(KT8[:, e, key0:key0 + 512], kps[kg])

            def proj_queries():
                # both slot groups; stationary W chunk shared across groups
                for e in range(8):
                    qps = [ps_mm.tile([128, 512], F32, tag="mm", name=f"q{e}_{g}")
                           for g in range(2)]
                    for c in range(8):
                        for g in range(2):
                            nc.tensor.matmul(
                                qps[g], wq_sb[:, c, e * 128:(e + 1) * 128],
                                QXT[:, g * 4:(g + 1) * 4, c, :],
                                start=(c == 0), stop=(c == 7),
                            )
                    for g in range(2):
                        nc.vector.tensor_copy(QT8[:, e, g * 512:(g + 1) * 512], qps[g])
                    nc.vector.tensor_copy(QTb[:, e, :], qps[0][:, 0:128])

            def vb_fix():
                # bf16 V for key tiles 0,1 (fixup slot); stationary x chunk
                # shared across e-halves
                for t in range(2):
                    vbp = [ps_mm.tile([128, 512], F32, tag="mm", name=f"vb{t}_{eh}")
                           for eh in range(2)]
                    for c in range(8):
                        for eh in range(2):
                            nc.tensor.matmul(
                                vbp[eh], xT0[:, t, c, :],
                                wvb_sb[:, c, eh * 512:(eh + 1) * 512],
                                start=(c == 0), stop=(c == 7),
                            )
                    for eh in range(2):
                        nc.vector.tensor_copy(
                            Vb[:, t, eh * 512:(eh + 1) * 512], vbp[eh])

            v_phase(x8a, 0)
            v_phase(x8b, 8)
            k_pass(xT0, 0, 0)
            k_pass(xT0, 0, 1)
            k_pass_shared(xT1, 1)
            proj_queries()
            vb_fix()

        # ---- attention: S^T per key tile, then AV with P^T stationary ----
        with ExitStack() as ph3:
            pt_pool = ph3.enter_context(tc.tile_pool(name="ptp", bufs=1))
            ps_st = ph3.enter_context(tc.tile_pool(name="ps_st", bufs=3, space="PSUM"))
            ps_o = ph3.enter_context(tc.tile_pool(name="ps_o", bufs=2, space="PSUM"))
            ps_rs = ph3.enter_context(tc.tile_pool(name="ps_rs", bufs=1, space="PSUM"))
            sc_pool = ph3.enter_context(tc.tile_pool(name="scp", bufs=2))
            outp = ph3.enter_context(tc.tile_pool(name="outp", bufs=2))

            PTs = [
                pt_pool.tile([128, 8, 512], F8, tag="pt1", name="PT1"),
                pt_pool.tile([128, 16, 512], F8, tag="pt2", name="PT2"),
            ]
            Pb = pt_pool.tile([128, 2, 128], BF16, tag="pb", name="Pb")

            def st_fused():
                # one pass over key tiles; each KT stationary chunk-pair
                # serves BOTH slot groups' S^T matmuls (kt<8)
                for kt in range(16):
                    work = []   # (group, sps, w, col0, f)
                    for g in ((1, 0) if kt < 8 else (1,)):
                        Ls = LIMITS[g * 4:(g + 1) * 4]
                        f = sum(1 for L in Ls if L <= kt)
                        w = (4 - f) * 128
                        col0 = f * 128
                        sps = ps_st.tile([128, 512], F32, tag="st",
                                         name=f"s{g}_{kt}")
                        work.append((g, sps, w, col0, f))
                    for c2 in range(4):
                        for g, sps, w, col0, f in work:
                            nc.tensor.matmul(
                                sps[:, 0:w],
                                KT8[:, 2 * c2:2 * c2 + 2, kt * 128:(kt + 1) * 128],
                                QT8[:, 2 * c2:2 * c2 + 2,
                                    g * 512 + col0: g * 512 + col0 + w],
                                start=(c2 == 0), stop=(c2 == 3),
                                perf_mode=DR,
                            )
                    for g, sps, w, col0, f in work:
                        Ls = LIMITS[g * 4:(g + 1) * 4]
                        if kt == Ls[f] - 2:
                            nc.vector.tensor_add(
                                sps[:, 0:128], sps[:, 0:128],
                                mask_sb[:, g * 256: g * 256 + 128],
                            )
                        elif kt == Ls[f] - 1:
                            nc.vector.tensor_add(
                                sps[:, 0:128], sps[:, 0:128],
                                mask_sb[:, g * 256 + 128: g * 256 + 256],
                            )
                        nc.scalar.activation(
                            PTs[g][:, kt, col0:col0 + w], sps[:, 0:w],
                            mybir.ActivationFunctionType.Exp,
                            bias=0.0, scale=EXP_SCALE,
                        )

            def st_fix():
                # bf16 S^T for the fixup slot (g0 col0, key tiles 0,1)
                for kt in range(2):
                    spb = ps_st.tile([128, 512], F32, tag="st", name=f"sf{kt}")
                    for c in range(8):
                        nc.tensor.matmul(
                            spb[:, 0:128], KTb[:, c, kt * 128:(kt + 1) * 128],
                            QTb[:, c, :],
                            start=(c == 0), stop=(c == 7),
                        )
                    nc.vector.tensor_add(
                        spb[:, 0:128], spb[:, 0:128],
                        mask_sb[:, kt * 128:(kt + 1) * 128],
                    )
                    nc.scalar.activation(
                        Pb[:, kt, :], spb[:, 0:128],
                        mybir.ActivationFunctionType.Exp,
                        bias=0.0, scale=EXP_SCALE,
                    )

            def av_epilogue(slot, O_ps, rs_ps):
                stats = sc_pool.tile([128, 8], F32, tag="stats", name=f"st{slot}")
                recip = stats[:, 0:1]
                nc.vector.reciprocal(recip, rs_ps)
                out_sb = outp.tile([128, D], BF16, tag="osb", name=f"ou{slot}")
                nc.vector.tensor_scalar_mul(out_sb, O_ps, recip)
                eng = nc.scalar if slot % 2 == 0 else nc.gpsimd
                eng.dma_start(out=out_q[slot][:, :], in_=out_sb)

            def av_slot(g, j):
                # fp8 DoubleRow over key-tile pairs; rowsum reuses stationary
                PT = PTs[g]
                slot = g * 4 + j
                L = LIMITS[slot]
                col = j * 128
                O_ps = ps_o.tile([128, D], F32, tag="O", name=f"O{slot}")
                rs_ps = ps_rs.tile([128, 1], F32, tag="rs", name=f"r{slot}")
                L2 = L // 2
                for t2 in range(L2):
                    pt_blk = PT[:, 2 * t2:2 * t2 + 2, col:col + 128]
                    for h in range(2):
                        nc.tensor.matmul(
                            O_ps[:, h * 512:(h + 1) * 512], pt_blk,
                            V8[:, 2 * t2:2 * t2 + 2, h * 512:(h + 1) * 512],
                            start=(t2 == 0), stop=(t2 == L2 - 1),
                            perf_mode=DR,
                        )
                    nc.tensor.matmul(
                        rs_ps, pt_blk, ones8[:, :, 0:1],
                        start=(t2 == 0), stop=(t2 == L2 - 1),
                        perf_mode=DR,
                    )
                av_epilogue(slot, O_ps, rs_ps)

            def av_fix():
                # bf16 AV for the fixup slot (slot 0, L=2)
                O_ps = ps_o.tile([128, D], F32, tag="O", name="Ofix")
                rs_ps = ps_rs.tile([128, 1], F32, tag="rs", name="rfix")
                for kt in range(2):
                    pb_blk = Pb[:, kt, :]
                    for h in range(2):
                        nc.tensor.matmul(
                            O_ps[:, h * 512:(h + 1) * 512], pb_blk,
                            Vb[:, kt, h * 512:(h + 1) * 512],
                            start=(kt == 0), stop=(kt == 1),
                        )
                    nc.tensor.matmul(
                        rs_ps, pb_blk, onesb[:, 0:1],
                        start=(kt == 0), stop=(kt == 1),
                    )
                av_epilogue(0, O_ps, rs_ps)

            # interleave big(g1)/small(g0) slots in descending L; fixup slot
            # (L=2, bf16) last so the end-of-kernel chain is shortest
            st_fused()
            st_fix()
            for j in (3, 2, 1):
                av_slot(1, j)
                av_slot(0, j)
            av_slot(1, 0)
            av_fix()

    nc.compile()
    return nc


def _masks():
    k = np.arange(128)[:, None]
    q = np.arange(128)[None, :]
    tril_t = np.where(k <= q, 0.0, NEG).astype(np.float32)  # S^T diag block
    fullneg = np.full((128, 128), NEG, np.float32)
    zeros = np.zeros((128, 128), np.float32)
    m_s0 = np.concatenate([tril_t, fullneg, zeros, tril_t], axis=1)
    m_s1 = np.concatenate([zeros, tril_t, tril_t, fullneg], axis=1)
    return m_s0, m_s1


def kernel(x, Wq, Wk, Wv):
    global LAST_EXEC_NS
    x = np.asarray(x, dtype=np.float32)
    Wq = np.asarray(Wq, dtype=np.float32)
    Wk = np.asarray(Wk, dtype=np.float32)
    Wv = np.asarray(Wv, dtype=np.float32)

    if "nc" not in _NC_CACHE:
        _NC_CACHE["nc"] = _build_nc()
    nc = _NC_CACHE["nc"]

    # host pre-transpose: x[b] (N, D) -> (tile, p=d%128, dchunk, token)
    xt_f32 = np.ascontiguousarray(
        x.reshape(B, N_KTILES, 128, 8, 128).transpose(0, 1, 4, 3, 2)
    )  # [B, tile, p, c, q] f32
    xt_all = xt_f32.astype(BF)
    x8_all = xt_f32.astype(F8NP)

    # weights -> [p=d%128, dchunk, ecol]; premuls folded in
    wq_r = np.ascontiguousarray(
        (QK_PREMUL * Wq).reshape(8, 128, 1024).transpose(1, 0, 2).astype(BF))
    wk_r = np.ascontiguousarray(
        (QK_PREMUL * Wk).reshape(8, 128, 1024).transpose(1, 0, 2).astype(BF))
    wv_scaled = np.ascontiguousarray(
        (V_PREMUL * Wv).reshape(8, 128, 1024).transpose(1, 0, 2))
    wv8_r = wv_scaled.astype(F8NP)
    wvb_r = wv_scaled.astype(BF)

    m_s0, m_s1 = _masks()
    in_maps = []
    for c in range(N_CORES):
        b, s = divmod(c, 2)
        in_maps.append({
            "x_t": xt_all[b],
            "x_t8": x8_all[b],
            "x_qt": np.ascontiguousarray(xt_all[b, QSETS[s]]),
            "wq": wq_r, "wk": wk_r, "wv8": wv8_r, "wvb": wvb_r,
            "mask": m_s1 if s else m_s0,
        })

    res = run_bass_kernel_spmd(nc, in_maps, list(range(N_CORES)), trace=TRACE)
    LAST_EXEC_NS = res.exec_time_ns

    out = np.empty((B, N, D), dtype=np.float32)
    for c in range(N_CORES):
        b, s = divmod(c, 2)
        oq = np.asarray(res.results[c]["out_q"], dtype=np.float32)
        for j, g in enumerate(QSETS[s]):
            out[b, g * 128:(g + 1) * 128, :] = oq[j]
    return out


# revision 5
# speedup vs baseline: 1.4319x; 1.1138x over previous
"""Causal attention (B=4, N=2048, D=1024) on 8 Trainium2 NeuronCores.

v4 design (vs v3 fp8, 183us):
  * Intra-pair K/V projection split: the two cores of a batch pair each
    project only THEIR 8 key tiles of K^T (bf16) and V (fp8 DoubleRow),
    then exchange halves with pair AllGather collectives
    (replica_groups [[0,1],[2,3],[4,5],[6,7]]) staged through DRAM
    bounce buffers on the gpsimd ring.  AllGather output index h is
    replica h's data, so the gathered tensor is in TRUE key order on
    both cores -- the program stays SPMD-uniform.  Saves 27.3us (K) +
    14.5us (V) of duplicated PE work per core; the exchange hides under
    the Q projection.
  * fp8(e4m3) DoubleRow matmuls for V proj, S^T and AV; Q/K stay bf16.
    Scales folded into host weights: wq,wk = 4*W; wv8 = e4m3(32*Wv);
    exp scale = (1/32)/16; rowsum ones = 32.0 cancels the V premul.
  * Early-row fixup: each core's L=2 slot (q-tile 0 for s=0, 1 for s=1)
    runs a full bf16 path (bf16 K^T/Q^T/P/V for true keys 0..255).  The
    bf16 KTb/Vb are computed from s=0's local tiles and broadcast via a
    small bf16 AllGather (s=1's contribution is ignored).  numpy-sim of
    the exact mix: max rel err 4.4e-3 (tolerance 2e-2).
  * All input DMAs on ONE logical queue (sync/HWDGE) in priority order:
    SDMA engines round-robin active queues at packet granularity, so a
    single queue is the only way to get true priority + full ~340 GB/s.
  * Core 2b+s handles batch b; s=0 takes query tiles {0,2,4,6, 9,11,13,15},
    s=1 takes {1,3,5,7, 8,10,12,14} -- both 68 causal key-tile pairs.
"""
import sys

sys.path.insert(0, "/opt/trn_rl_repo")

from contextlib import ExitStack

import numpy as np
import ml_dtypes

import concourse.bass as bass
import concourse.mybir as mybir
import concourse.tile as tile
from concourse import bacc
from concourse.bass_utils import run_bass_kernel_spmd

B, N, D = 4, 2048, 1024
N_CORES = 8
N_SLOTS = 8
N_KTILES = 16
SCALE = 1.0 / 32.0   # 1/sqrt(D)
QK_PREMUL = 4.0      # folded into wq/wk on host
V_PREMUL = 32.0      # folded into wv on host
EXP_SCALE = SCALE / (QK_PREMUL * QK_PREMUL)
NEG = -1.0e9

F32 = mybir.dt.float32
BF16 = mybir.dt.bfloat16
F8 = mybir.dt.float8e4
DR = mybir.MatmulPerfMode.DoubleRow
BF = ml_dtypes.bfloat16
F8NP = ml_dtypes.float8_e4m3

PAIRS = [[0, 1], [2, 3], [4, 5], [6, 7]]

# query-tile sets per parity slot s (ascending); both have sum(g+1) == 68
QSETS = [
    [0, 2, 4, 6, 9, 11, 13, 15],
    [1, 3, 5, 7, 8, 10, 12, 14],
]
# uniform program limits per slot (key tiles 0..L-1 computed)
LIMITS = [2, 4, 6, 8, 10, 12, 14, 16]

_NC_CACHE = {}
TRACE = False
LAST_EXEC_NS = None


def _build_nc():
    nc = bacc.Bacc(None, target_bir_lowering=False, debug=False, num_devices=8)

    # x tile layouts: [tile, p=d%128, dchunk, token]; _kt = own key half
    x_kt = nc.declare_dram_parameter("x_kt", [8, 128, 8, 128], BF16, isOutput=False)
    x_kt8 = nc.declare_dram_parameter("x_kt8", [8, 128, 8, 128], F8, isOutput=False)
    x_qt = nc.declare_dram_parameter("x_qt", [N_SLOTS, 128, 8, 128], BF16, isOutput=False)
    # weights: [p=d%128, dchunk, ecol]
    wq = nc.declare_dram_parameter("wq", [128, 8, 1024], BF16, isOutput=False)
    wk = nc.declare_dram_parameter("wk", [128, 8, 1024], BF16, isOutput=False)
    wv8 = nc.declare_dram_parameter("wv8", [128, 8, 1024], F8, isOutput=False)
    wvb = nc.declare_dram_parameter("wvb", [128, 8, 1024], BF16, isOutput=False)
    mask_in = nc.declare_dram_parameter("mask", [128, 512], F32, isOutput=False)
    out_q = nc.declare_dram_parameter("out_q", [N_SLOTS, 128, D], BF16, isOutput=True)

    with tile.TileContext(nc) as tc, ExitStack() as top:
        consts = top.enter_context(tc.tile_pool(name="consts", bufs=1))
        kt_pool = top.enter_context(tc.tile_pool(name="ktp", bufs=1))
        v_pool = top.enter_context(tc.tile_pool(name="vp", bufs=1))
        qt_pool = top.enter_context(tc.tile_pool(name="qtp", bufs=1))
        dram = top.enter_context(tc.tile_pool(name="dram", bufs=6, space="DRAM"))

        ones8 = consts.tile([128, 2, 16], F8)
        nc.vector.memset(ones8, V_PREMUL)
        onesb = consts.tile([128, 8], BF16)
        nc.vector.memset(onesb, V_PREMUL)
        mask_sb = consts.tile([128, 512], F32)

        KT8 = kt_pool.tile([128, 8, N], F8)        # [p=e%128, echunk, key]
        KTb = kt_pool.tile([128, 8, 256], BF16)    # bf16 keys 0..255 (fixup)
        QT8 = qt_pool.tile([128, 8, 1024], F8)     # [p=e%128, echunk, qcol]
        QTb = qt_pool.tile([128, 8, 128], BF16)    # bf16 fixup q-tile (g0 col 0)
        V8 = v_pool.tile([128, N_KTILES, D], F8)   # [p=key%128, ktile, e]
        Vb = v_pool.tile([128, 2, D], BF16)        # bf16 V kt0/1 (fixup)

        # DRAM bounce buffers for the pair exchanges
        stV = dram.tile([128, 8, 1024], F8)        # own V half
        gV = dram.tile([2, 128, 8, 1024], F8)
        stK = dram.tile([128, 8, 1024], F8)        # own K^T half
        gK = dram.tile([2, 128, 8, 1024], F8)
        stb = dram.tile([128, 4, 1024], BF16)      # KTb (0:2) + Vb (2:4)
        gb = dram.tile([2, 128, 4, 1024], BF16)

        with ExitStack() as ph12:
            x8_pool = ph12.enter_context(tc.tile_pool(name="x8p", bufs=1))
            xt_pool = ph12.enter_context(tc.tile_pool(name="xtp", bufs=1))
            qxt_pool = ph12.enter_context(tc.tile_pool(name="qxt", bufs=1))
            w_pool = ph12.enter_context(tc.tile_pool(name="wp", bufs=1))
            hf_pool = ph12.enter_context(tc.tile_pool(name="hf", bufs=1))
            ps_mm = ph12.enter_context(tc.tile_pool(name="ps_mm", bufs=8, space="PSUM"))

            # ---- input DMAs: ONE queue (sync/HWDGE), strict priority ----
            wv8_sb = w_pool.tile([128, 8, 1024], F8, tag="wv8")
            x8 = x8_pool.tile([128, 8, 8, 128], F8, tag="x8")
            wk_sb = w_pool.tile([128, 8, 1024], BF16, tag="wk")
            xT = xt_pool.tile([128, 8, 8, 128], BF16, tag="xT")
            wvb_sb = w_pool.tile([128, 8, 1024], BF16, tag="wvb")
            QXT = qxt_pool.tile([128, 8, 8, 128], BF16, tag="qx")
            wq_sb = w_pool.tile([128, 8, 1024], BF16, tag="wq")

            nc.sync.dma_start(out=wv8_sb[:, 0:2, :], in_=wv8[:, 0:2, :])
            nc.sync.dma_start(
                out=x8[:, 0:2], in_=x_kt8[0:2].rearrange("t p c q -> p t c q"))
            nc.sync.dma_start(out=wv8_sb[:, 2:8, :], in_=wv8[:, 2:8, :])
            nc.sync.dma_start(
                out=x8[:, 2:8], in_=x_kt8[2:8].rearrange("t p c q -> p t c q"))
            nc.sync.dma_start(out=wk_sb, in_=wk[:, :, :])
            nc.sync.dma_start(
                out=xT[:, 0:4], in_=x_kt[0:4].rearrange("t p c q -> p t c q"))
            nc.sync.dma_start(
                out=xT[:, 4:8], in_=x_kt[4:8].rearrange("t p c q -> p t c q"))
            nc.sync.dma_start(out=wvb_sb, in_=wvb[:, :, :])
            nc.sync.dma_start(
                out=QXT, in_=x_qt[:].rearrange("s p c q -> p s c q"))
            nc.sync.dma_start(out=wq_sb, in_=wq[:, :, :])
            nc.sync.dma_start(out=mask_sb, in_=mask_in[:, :])

            vhalf = hf_pool.tile([128, 8, 1024], F8, tag="vh")
            khalf = hf_pool.tile([128, 8, 1024], F8, tag="kh")
            ktb_sb = hf_pool.tile([128, 8, 256], BF16, tag="ktb")
            vb_sb = hf_pool.tile([128, 2, 1024], BF16, tag="vbs")

            def v_half():
                # fp8 DoubleRow; stationary x chunk-pair shared by both e-halves
                for lt in range(8):
                    vps = [ps_mm.tile([128, 512], F32, tag="mm", name=f"v{lt}_{eh}")
                           for eh in range(2)]
                    for c2 in range(4):
                        for eh in range(2):
                            nc.tensor.matmul(
                                vps[eh],
                                x8[:, lt, 2 * c2:2 * c2 + 2, :],
                                wv8_sb[:, 2 * c2:2 * c2 + 2, eh * 512:(eh + 1) * 512],
                                start=(c2 == 0), stop=(c2 == 3),
                                perf_mode=DR,
                            )
                    for eh in range(2):
                        nc.vector.tensor_copy(
                            vhalf[:, lt, eh * 512:(eh + 1) * 512], vps[eh])

            def k_pass(tg):
                # bf16 K^T projection for one 4-tile group of the own half
                for e in range(8):
                    kps = ps_mm.tile([128, 512], F32, tag="mm", name=f"k{tg}_{e}")
                    for c in range(8):
                        nc.tensor.matmul(
                            kps, wk_sb[:, c, e * 128:(e + 1) * 128],
                            xT[:, tg * 4:(tg + 1) * 4, c, :],
                            start=(c == 0), stop=(c == 7),
                        )
                    nc.vector.tensor_copy(khalf[:, e, tg * 512:(tg + 1) * 512], kps)
                    if tg == 0:
                        nc.vector.tensor_copy(ktb_sb[:, e, :], kps[:, 0:256])

            def vb_fix():
                # bf16 V for local tiles 0,1 (true kt0/1 on s=0 cores; the
                # gather takes replica 0's data, s=1's result is discarded)
                for t in range(2):
                    vbp = [ps_mm.tile([128, 512], F32, tag="mm", name=f"vb{t}_{eh}")
                           for eh in range(2)]
                    for c in range(8):
                        for eh in range(2):
                            nc.tensor.matmul(
                                vbp[eh], xT[:, t, c, :],
                                wvb_sb[:, c, eh * 512:(eh + 1) * 512],
                                start=(c == 0), stop=(c == 7),
                            )
                    for eh in range(2):
                        nc.vector.tensor_copy(
                            vb_sb[:, t, eh * 512:(eh + 1) * 512], vbp[eh])

            def proj_queries():
                # both slot groups; stationary W chunk shared across groups
                for e in range(8):
                    qps = [ps_mm.tile([128, 512], F32, tag="mm", name=f"q{e}_{g}")
                           for g in range(2)]
                    for c in range(8):
                        for g in range(2):
                            nc.tensor.matmul(
                                qps[g], wq_sb[:, c, e * 128:(e + 1) * 128],
                                QXT[:, g * 4:(g + 1) * 4, c, :],
                                start=(c == 0), stop=(c == 7),
                            )
                    for g in range(2):
                        nc.vector.tensor_copy(QT8[:, e, g * 512:(g + 1) * 512], qps[g])
                    nc.vector.tensor_copy(QTb[:, e, :], qps[0][:, 0:128])

            # --- projections + pair exchange (collectives on gpsimd ring) ---
            v_half()
            nc.gpsimd.dma_start(out=stV[:], in_=vhalf)
            nc.gpsimd.collective_compute(
                "AllGather", mybir.AluOpType.bypass, replica_groups=PAIRS,
                ins=[stV.opt()], outs=[gV.opt()])
            k_pass(0)
            k_pass(1)
            nc.gpsimd.dma_start(out=stK[:], in_=khalf)
            nc.gpsimd.collective_compute(
                "AllGather", mybir.AluOpType.bypass, replica_groups=PAIRS,
                ins=[stK.opt()], outs=[gK.opt()])
            # readbacks (true key order: gather index h = replica h's half)
            for h in range(2):
                nc.gpsimd.dma_start(
                    out=V8[:, h * 8:(h + 1) * 8, :], in_=gV[h][:, :, :])
                nc.gpsimd.dma_start(
                    out=KT8[:, :, h * 1024:(h + 1) * 1024], in_=gK[h][:, :, :])
            vb_fix()
            nc.gpsimd.dma_start(
                out=stb[:, 0:2, :].rearrange("p a b -> p (a b)"),
                in_=ktb_sb.rearrange("p a b -> p (a b)"))
            nc.gpsimd.dma_start(out=stb[:, 2:4, :], in_=vb_sb)
            nc.gpsimd.collective_compute(
                "AllGather", mybir.AluOpType.bypass, replica_groups=PAIRS,
                ins=[stb.opt()], outs=[gb.opt()])
            nc.gpsimd.dma_start(
                out=KTb.rearrange("p a b -> p (a b)"),
                in_=gb[0][:, 0:2, :].rearrange("p a b -> p (a b)"))
            nc.gpsimd.dma_start(out=Vb, in_=gb[0][:, 2:4, :])
            proj_queries()

        # ---- attention: S^T per key tile, then AV with P^T stationary ----
        with ExitStack() as ph3:
            pt_pool = ph3.enter_context(tc.tile_pool(name="ptp", bufs=1))
            sc_pool = ph3.enter_context(tc.tile_pool(name="scp", bufs=2))
            outp = ph3.enter_context(tc.tile_pool(name="outp", bufs=2))

            PTs = [
                pt_pool.tile([128, 8, 512], F8, tag="pt1", name="PT1"),
                pt_pool.tile([128, 16, 512], F8, tag="pt2", name="PT2"),
            ]
            Pb = pt_pool.tile([128, 2, 128], BF16, tag="pb", name="Pb")

            def st_fused(ps_st):
                # one pass over key tiles; each KT stationary chunk-pair
                # serves BOTH slot groups' S^T matmuls (kt<8)
                for kt in range(16):
                    work = []   # (group, sps, w, col0, f)
                    for g in ((1, 0) if kt < 8 else (1,)):
                        Ls = LIMITS[g * 4:(g + 1) * 4]
                        f = sum(1 for L in Ls if L <= kt)
                        w = (4 - f) * 128
                        col0 = f * 128
                        sps = ps_st.tile([128, 512], F32, tag="st",
                                         name=f"s{g}_{kt}")
                        work.append((g, sps, w, col0, f))
                    for c2 in range(4):
                        for g, sps, w, col0, f in work:
                            nc.tensor.matmul(
                                sps[:, 0:w],
                                KT8[:, 2 * c2:2 * c2 + 2, kt * 128:(kt + 1) * 128],
                                QT8[:, 2 * c2:2 * c2 + 2,
                                    g * 512 + col0: g * 512 + col0 + w],
                                start=(c2 == 0), stop=(c2 == 3),
                                perf_mode=DR,
                            )
                    for g, sps, w, col0, f in work:
                        Ls = LIMITS[g * 4:(g + 1) * 4]
                        if kt == Ls[f] - 2:
                            nc.vector.tensor_add(
                                sps[:, 0:128], sps[:, 0:128],
                                mask_sb[:, g * 256: g * 256 + 128],
                            )
                        elif kt == Ls[f] - 1:
                            nc.vector.tensor_add(
                                sps[:, 0:128], sps[:, 0:128],
                                mask_sb[:, g * 256 + 128: g * 256 + 256],
                            )
                        nc.scalar.activation(
                            PTs[g][:, kt, col0:col0 + w], sps[:, 0:w],
                            mybir.ActivationFunctionType.Exp,
                            bias=0.0, scale=EXP_SCALE,
                        )

            def st_fix(ps_st):
                # bf16 S^T for the fixup slot (g0 col0, true key tiles 0,1)
                for kt in range(2):
                    spb = ps_st.tile([128, 512], F32, tag="st", name=f"sf{kt}")
                    for c in range(8):
                        nc.tensor.matmul(
                            spb[:, 0:128], KTb[:, c, kt * 128:(kt + 1) * 128],
                            QTb[:, c, :],
                            start=(c == 0), stop=(c == 7),
                        )
                    nc.vector.tensor_add(
                        spb[:, 0:128], spb[:, 0:128],
                        mask_sb[:, kt * 128:(kt + 1) * 128],
                    )
                    nc.scalar.activation(
                        Pb[:, kt, :], spb[:, 0:128],
                        mybir.ActivationFunctionType.Exp,
                        bias=0.0, scale=EXP_SCALE,
                    )

            with ExitStack() as st_scope:
                ps_st = st_scope.enter_context(
                    tc.tile_pool(name="ps_st", bufs=3, space="PSUM"))
                st_fused(ps_st)
                st_fix(ps_st)

            ps_o = ph3.enter_context(tc.tile_pool(name="ps_o", bufs=3, space="PSUM"))
            ps_rs = ph3.enter_context(tc.tile_pool(name="ps_rs", bufs=2, space="PSUM"))

            def av_epilogue(slot, O_ps, rs_ps):
                stats = sc_pool.tile([128, 8], F32, tag="stats", name=f"st{slot}")
                recip = stats[:, 0:1]
                nc.vector.reciprocal(recip, rs_ps)
                out_sb = outp.tile([128, D], BF16, tag="osb", name=f"ou{slot}")
                nc.vector.tensor_scalar_mul(out_sb, O_ps, recip)
                eng = nc.scalar if slot % 2 == 0 else nc.gpsimd
                eng.dma_start(out=out_q[slot][:, :], in_=out_sb)

            def av_slot(g, j):
                # fp8 DoubleRow over key-tile pairs; rowsum reuses stationary
                PT = PTs[g]
                slot = g * 4 + j
                L = LIMITS[slot]
                col = j * 128
                O_ps = ps_o.tile([128, D], F32, tag="O", name=f"O{slot}")
                rs_ps = ps_rs.tile([128, 1], F32, tag="rs", name=f"r{slot}")
                L2 = L // 2
                for t2 in range(L2):
                    pt_blk = PT[:, 2 * t2:2 * t2 + 2, col:col + 128]
                    for h in range(2):
                        nc.tensor.matmul(
                            O_ps[:, h * 512:(h + 1) * 512], pt_blk,
                            V8[:, 2 * t2:2 * t2 + 2, h * 512:(h + 1) * 512],
                            start=(t2 == 0), stop=(t2 == L2 - 1),
                            perf_mode=DR,
                        )
                    nc.tensor.matmul(
                        rs_ps, pt_blk, ones8[:, :, 0:1],
                        start=(t2 == 0), stop=(t2 == L2 - 1),
                        perf_mode=DR,
                    )
                av_epilogue(slot, O_ps, rs_ps)

            def av_fix():
                # bf16 AV for the fixup slot (slot 0, L=2)
                O_ps = ps_o.tile([128, D], F32, tag="O", name="Ofix")
                rs_ps = ps_rs.tile([128, 1], F32, tag="rs", name="rfix")
                for kt in range(2):
                    pb_blk = Pb[:, kt, :]
                    for h in range(2):
                        nc.tensor.matmul(
                            O_ps[:, h * 512:(h + 1) * 512], pb_blk,
                            Vb[:, kt, h * 512:(h + 1) * 512],
                            start=(kt == 0), stop=(kt == 1),
                        )
                    nc.tensor.matmul(
                        rs_ps, pb_blk, onesb[:, 0:1],
                        start=(kt == 0), stop=(kt == 1),
                    )
                av_epilogue(0, O_ps, rs_ps)

            # interleave big(g1)/small(g0) slots in descending L; fixup slot
            # (L=2, bf16) last so the end-of-kernel chain is shortest
            for j in (3, 2, 1):
                av_slot(1, j)
                av_slot(0, j)
            av_slot(1, 0)
            av_fix()

    nc.compile()
    return nc


def _masks():
    k = np.arange(128)[:, None]
    q = np.arange(128)[None, :]
    tril_t = np.where(k <= q, 0.0, NEG).astype(np.float32)  # S^T diag block
    fullneg = np.full((128, 128), NEG, np.float32)
    zeros = np.zeros((128, 128), np.float32)
    m_s0 = np.concatenate([tril_t, fullneg, zeros, tril_t], axis=1)
    m_s1 = np.concatenate([zeros, tril_t, tril_t, fullneg], axis=1)
    return m_s0, m_s1


def kernel(x, Wq, Wk, Wv):
    global LAST_EXEC_NS
    x = np.asarray(x, dtype=np.float32)
    Wq = np.asarray(Wq, dtype=np.float32)
    Wk = np.asarray(Wk, dtype=np.float32)
    Wv = np.asarray(Wv, dtype=np.float32)

    if "nc" not in _NC_CACHE:
        _NC_CACHE["nc"] = _build_nc()
    nc = _NC_CACHE["nc"]

    # host pre-transpose: x[b] (N, D) -> (tile, p=d%128, dchunk, token)
    xt_f32 = np.ascontiguousarray(
        x.reshape(B, N_KTILES, 128, 8, 128).transpose(0, 1, 4, 3, 2)
    )  # [B, tile, p, c, q] f32
    xt_all = xt_f32.astype(BF)
    x8_all = xt_f32.astype(F8NP)

    # weights -> [p=d%128, dchunk, ecol]; premuls folded in
    wq_r = np.ascontiguousarray(
        (QK_PREMUL * Wq).reshape(8, 128, 1024).transpose(1, 0, 2).astype(BF))
    wk_r = np.ascontiguousarray(
        (QK_PREMUL * Wk).reshape(8, 128, 1024).transpose(1, 0, 2).astype(BF))
    wv_scaled = np.ascontiguousarray(
        (V_PREMUL * Wv).reshape(8, 128, 1024).transpose(1, 0, 2))
    wv8_r = wv_scaled.astype(F8NP)
    wvb_r = wv_scaled.astype(BF)

    m_s0, m_s1 = _masks()
    in_maps = []
    for c in range(N_CORES):
        b, s = divmod(c, 2)
        in_maps.append({
            "x_kt": np.ascontiguousarray(xt_all[b, s * 8:(s + 1) * 8]),
            "x_kt8": np.ascontiguousarray(x8_all[b, s * 8:(s + 1) * 8]),
            "x_qt": np.ascontiguousarray(xt_all[b, QSETS[s]]),
            "wq": wq_r, "wk": wk_r, "wv8": wv8_r, "wvb": wvb_r,
            "mask": m_s1 if s else m_s0,
        })

    res = run_bass_kernel_spmd(nc, in_maps, list(range(N_CORES)), trace=TRACE)
    LAST_EXEC_NS = res.exec_time_ns

    out = np.empty((B, N, D), dtype=np.float32)
    for c in range(N_CORES):
        b, s = divmod(c, 2)
        oq = np.asarray(res.results[c]["out_q"], dtype=np.float32)
        for j, g in enumerate(QSETS[s]):
            out[b, g * 128:(g + 1) * 128, :] = oq[j]
    return out


# revision 9
# speedup vs baseline: 1.5711x; 1.0972x over previous
"""Causal attention (B=4, N=2048, D=1024) on 8 Trainium2 NeuronCores.

v4 design (vs v3 fp8, 183us):
  * Intra-pair K/V projection split: the two cores of a batch pair each
    project only THEIR 8 key tiles of K^T (bf16) and V (fp8 DoubleRow),
    then exchange halves with pair AllGather collectives
    (replica_groups [[0,1],[2,3],[4,5],[6,7]]) staged through DRAM
    bounce buffers on the gpsimd ring.  AllGather output index h is
    replica h's data, so the gathered tensor is in TRUE key order on
    both cores -- the program stays SPMD-uniform.  Saves 27.3us (K) +
    14.5us (V) of duplicated PE work per core; the exchange hides under
    the Q projection.
  * fp8(e4m3) DoubleRow matmuls for V proj, S^T and AV; Q/K stay bf16.
    Scales folded into host weights: wq,wk = 4*W; wv8 = e4m3(32*Wv);
    exp scale = (1/32)/16; rowsum ones = 32.0 cancels the V premul.
  * Early-row fixup: each core's L=2 slot (q-tile 0 for s=0, 1 for s=1)
    runs a full bf16 path (bf16 K^T/Q^T/P/V for true keys 0..255).  The
    bf16 KTb/Vb are computed from s=0's local tiles and broadcast via a
    small bf16 AllGather (s=1's contribution is ignored).  numpy-sim of
    the exact mix: max rel err 4.4e-3 (tolerance 2e-2).
  * All input DMAs on ONE logical queue (sync/HWDGE) in priority order:
    SDMA engines round-robin active queues at packet granularity, so a
    single queue is the only way to get true priority + full ~340 GB/s.
  * Core 2b+s handles batch b; s=0 takes query tiles {0,2,4,6, 9,11,13,15},
    s=1 takes {1,3,5,7, 8,10,12,14} -- both 68 causal key-tile pairs.
"""
import sys

sys.path.insert(0, "/opt/trn_rl_repo")

from contextlib import ExitStack

import numpy as np
import ml_dtypes

import concourse.bass as bass
import concourse.mybir as mybir
import concourse.tile as tile
from concourse import bacc
from concourse.bass_utils import run_bass_kernel_spmd

B, N, D = 4, 2048, 1024
N_CORES = 8
N_SLOTS = 8
N_KTILES = 16
SCALE = 1.0 / 32.0   # 1/sqrt(D)
QK_PREMUL = 4.0      # folded into wq/wk on host
V_PREMUL = 32.0      # folded into wv on host
EXP_SCALE = SCALE / (QK_PREMUL * QK_PREMUL)
NEG = -1.0e9

F32 = mybir.dt.float32
BF16 = mybir.dt.bfloat16
F8 = mybir.dt.float8e4
DR = mybir.MatmulPerfMode.DoubleRow
BF = ml_dtypes.bfloat16
F8NP = ml_dtypes.float8_e4m3

PAIRS = [[0, 1], [2, 3], [4, 5], [6, 7]]

# query-tile sets per parity slot s (ascending); both have sum(g+1) == 68
QSETS = [
    [0, 2, 4, 6, 9, 11, 13, 15],
    [1, 3, 5, 7, 8, 10, 12, 14],
]
# uniform program limits per slot (key tiles 0..L-1 computed)
LIMITS = [2, 4, 6, 8, 10, 12, 14, 16]

_NC_CACHE = {}
TRACE = False
LAST_EXEC_NS = None


def _build_nc():
    nc = bacc.Bacc(None, target_bir_lowering=False, debug=False, num_devices=8)

    # x tile layouts: [tile, p=d%128, dchunk, token]; _kt = own key half
    x_kt = nc.declare_dram_parameter("x_kt", [8, 128, 8, 128], BF16, isOutput=False)
    x_kt8 = nc.declare_dram_parameter("x_kt8", [8, 128, 8, 128], F8, isOutput=False)
    x_qt = nc.declare_dram_parameter("x_qt", [N_SLOTS, 128, 8, 128], BF16, isOutput=False)
    # weights: [p=d%128, dchunk, ecol]
    wq = nc.declare_dram_parameter("wq", [128, 8, 1024], BF16, isOutput=False)
    wk = nc.declare_dram_parameter("wk", [128, 8, 1024], BF16, isOutput=False)
    wv8 = nc.declare_dram_parameter("wv8", [128, 8, 1024], F8, isOutput=False)
    wvb = nc.declare_dram_parameter("wvb", [128, 8, 1024], BF16, isOutput=False)
    mask_in = nc.declare_dram_parameter("mask", [128, 512], F32, isOutput=False)
    out_q = nc.declare_dram_parameter("out_q", [N_SLOTS, 128, D], BF16, isOutput=True)

    with tile.TileContext(nc) as tc, ExitStack() as top:
        consts = top.enter_context(tc.tile_pool(name="consts", bufs=1))
        kt_pool = top.enter_context(tc.tile_pool(name="ktp", bufs=1))
        v_pool = top.enter_context(tc.tile_pool(name="vp", bufs=1))
        qt_pool = top.enter_context(tc.tile_pool(name="qtp", bufs=1))
        dram = top.enter_context(tc.tile_pool(name="dram", bufs=6, space="DRAM"))

        ones8 = consts.tile([128, 2, 16], F8)
        nc.vector.memset(ones8, V_PREMUL)
        onesb = consts.tile([128, 8], BF16)
        nc.vector.memset(onesb, V_PREMUL)
        mask_sb = consts.tile([128, 512], F32)

        KT8 = kt_pool.tile([128, 8, N], F8)        # [p=e%128, echunk, key]
        KTb = kt_pool.tile([128, 8, 256], BF16)    # bf16 keys 0..255 (fixup)
        QT8 = qt_pool.tile([128, 8, 1024], F8)     # [p=e%128, echunk, qcol]
        QTb = qt_pool.tile([128, 8, 128], BF16)    # bf16 fixup q-tile (g0 col 0)
        V8 = v_pool.tile([128, N_KTILES, D], F8)   # [p=key%128, ktile, e]
        Vb = v_pool.tile([128, 2, D], BF16)        # bf16 V kt0/1 (fixup)

        # DRAM bounce buffers for the pair exchanges
        stV = dram.tile([128, 8, 1024], F8)        # own V half
        gV = dram.tile([2, 128, 8, 1024], F8)
        stK = dram.tile([128, 8, 1024], F8)        # own K^T half
        gK = dram.tile([2, 128, 8, 1024], F8)
        stb = dram.tile([128, 4, 1024], BF16)      # KTb (0:2) + Vb (2:4)
        gb = dram.tile([2, 128, 4, 1024], BF16)

        with ExitStack() as ph12:
            x8_pool = ph12.enter_context(tc.tile_pool(name="x8p", bufs=1))
            xt_pool = ph12.enter_context(tc.tile_pool(name="xtp", bufs=1))
            qxt_pool = ph12.enter_context(tc.tile_pool(name="qxt", bufs=1))
            w_pool = ph12.enter_context(tc.tile_pool(name="wp", bufs=1))
            hf_pool = ph12.enter_context(tc.tile_pool(name="hf", bufs=1))
            ps_mm = ph12.enter_context(tc.tile_pool(name="ps_mm", bufs=8, space="PSUM"))

            # ---- input DMAs: ONE queue (sync/HWDGE), strict priority ----
            wv8_sb = w_pool.tile([128, 8, 1024], F8, tag="wv8")
            x8 = x8_pool.tile([128, 8, 8, 128], F8, tag="x8")
            wk_sb = w_pool.tile([128, 8, 1024], BF16, tag="wk")
            xT = xt_pool.tile([128, 8, 8, 128], BF16, tag="xT")
            wvb_sb = w_pool.tile([128, 8, 1024], BF16, tag="wvb")
            QXT = qxt_pool.tile([128, 8, 8, 128], BF16, tag="qx")
            wq_sb = w_pool.tile([128, 8, 1024], BF16, tag="wq")

            nc.sync.dma_start(out=wv8_sb[:, 0:2, :], in_=wv8[:, 0:2, :])
            nc.sync.dma_start(
                out=x8[:, 0:2], in_=x_kt8[0:2].rearrange("t p c q -> p t c q"))
            nc.sync.dma_start(out=wv8_sb[:, 2:8, :], in_=wv8[:, 2:8, :])
            nc.sync.dma_start(
                out=x8[:, 2:8], in_=x_kt8[2:8].rearrange("t p c q -> p t c q"))
            nc.sync.dma_start(out=wk_sb, in_=wk[:, :, :])
            nc.sync.dma_start(
                out=xT[:, 0:4], in_=x_kt[0:4].rearrange("t p c q -> p t c q"))
            nc.sync.dma_start(
                out=xT[:, 4:8], in_=x_kt[4:8].rearrange("t p c q -> p t c q"))
            nc.sync.dma_start(out=wvb_sb, in_=wvb[:, :, :])
            nc.sync.dma_start(
                out=QXT, in_=x_qt[:].rearrange("s p c q -> p s c q"))
            nc.sync.dma_start(out=wq_sb, in_=wq[:, :, :])
            nc.sync.dma_start(out=mask_sb, in_=mask_in[:, :])

            vhalf = hf_pool.tile([128, 8, 1024], F8, tag="vh")
            khalf = hf_pool.tile([128, 8, 1024], F8, tag="kh")
            ktb_sb = hf_pool.tile([128, 8, 256], BF16, tag="ktb")
            vb_sb = hf_pool.tile([128, 2, 1024], BF16, tag="vbs")

            def v_half():
                # fp8 DoubleRow; stationary x chunk-pair shared by both e-halves
                for lt in range(8):
                    vps = [ps_mm.tile([128, 512], F32, tag="mm", name=f"v{lt}_{eh}")
                           for eh in range(2)]
                    for c2 in range(4):
                        for eh in range(2):
                            nc.tensor.matmul(
                                vps[eh],
                                x8[:, lt, 2 * c2:2 * c2 + 2, :],
                                wv8_sb[:, 2 * c2:2 * c2 + 2, eh * 512:(eh + 1) * 512],
                                start=(c2 == 0), stop=(c2 == 3),
                                perf_mode=DR,
                            )
                    for eh in range(2):
                        nc.vector.tensor_copy(
                            vhalf[:, lt, eh * 512:(eh + 1) * 512], vps[eh])

            def k_pass(tg):
                # bf16 K^T projection for one 4-tile group of the own half
                for e in range(8):
                    kps = ps_mm.tile([128, 512], F32, tag="mm", name=f"k{tg}_{e}")
                    for c in range(8):
                        nc.tensor.matmul(
                            kps, wk_sb[:, c, e * 128:(e + 1) * 128],
                            xT[:, tg * 4:(tg + 1) * 4, c, :],
                            start=(c == 0), stop=(c == 7),
                        )
                    nc.vector.tensor_copy(khalf[:, e, tg * 512:(tg + 1) * 512], kps)
                    if tg == 0:
                        nc.vector.tensor_copy(ktb_sb[:, e, :], kps[:, 0:256])

            def vb_fix():
                # bf16 V for local tiles 0,1 (true kt0/1 on s=0 cores; the
                # gather takes replica 0's data, s=1's result is discarded)
                for t in range(2):
                    vbp = [ps_mm.tile([128, 512], F32, tag="mm", name=f"vb{t}_{eh}")
                           for eh in range(2)]
                    for c in range(8):
                        for eh in range(2):
                            nc.tensor.matmul(
                                vbp[eh], xT[:, t, c, :],
                                wvb_sb[:, c, eh * 512:(eh + 1) * 512],
                                start=(c == 0), stop=(c == 7),
                            )
                    for eh in range(2):
                        nc.vector.tensor_copy(
                            vb_sb[:, t, eh * 512:(eh + 1) * 512], vbp[eh])

            def proj_queries():
                # both slot groups; stationary W chunk shared across groups
                for e in range(8):
                    qps = [ps_mm.tile([128, 512], F32, tag="mm", name=f"q{e}_{g}")
                           for g in range(2)]
                    for c in range(8):
                        for g in range(2):
                            nc.tensor.matmul(
                                qps[g], wq_sb[:, c, e * 128:(e + 1) * 128],
                                QXT[:, g * 4:(g + 1) * 4, c, :],
                                start=(c == 0), stop=(c == 7),
                            )
                    for g in range(2):
                        nc.vector.tensor_copy(QT8[:, e, g * 512:(g + 1) * 512], qps[g])
                    nc.vector.tensor_copy(QTb[:, e, :], qps[0][:, 0:128])

            # --- projections + pair exchange (collectives on gpsimd ring) ---
            v_half()
            nc.gpsimd.dma_start(out=stV[:], in_=vhalf)
            nc.gpsimd.collective_compute(
                "AllGather", mybir.AluOpType.bypass, replica_groups=PAIRS,
                ins=[stV.opt()], outs=[gV.opt()])
            k_pass(0)
            k_pass(1)
            nc.gpsimd.dma_start(out=stK[:], in_=khalf)
            nc.gpsimd.collective_compute(
                "AllGather", mybir.AluOpType.bypass, replica_groups=PAIRS,
                ins=[stK.opt()], outs=[gK.opt()])
            # readbacks (true key order: gather index h = replica h's half)
            for h in range(2):
                nc.gpsimd.dma_start(
                    out=V8[:, h * 8:(h + 1) * 8, :], in_=gV[h][:, :, :])
                nc.gpsimd.dma_start(
                    out=KT8[:, :, h * 1024:(h + 1) * 1024], in_=gK[h][:, :, :])
            vb_fix()
            nc.gpsimd.dma_start(
                out=stb[:, 0:2, :].rearrange("p a b -> p (a b)"),
                in_=ktb_sb.rearrange("p a b -> p (a b)"))
            nc.gpsimd.dma_start(out=stb[:, 2:4, :], in_=vb_sb)
            nc.gpsimd.collective_compute(
                "AllGather", mybir.AluOpType.bypass, replica_groups=PAIRS,
                ins=[stb.opt()], outs=[gb.opt()])
            nc.gpsimd.dma_start(
                out=KTb.rearrange("p a b -> p (a b)"),
                in_=gb[0][:, 0:2, :].rearrange("p a b -> p (a b)"))
            nc.gpsimd.dma_start(out=Vb, in_=gb[0][:, 2:4, :])
            proj_queries()

        # ---- attention: S^T per key tile, then AV with P^T stationary ----
        with ExitStack() as ph3:
            pt_pool = ph3.enter_context(tc.tile_pool(name="ptp", bufs=1))
            sc_pool = ph3.enter_context(tc.tile_pool(name="scp", bufs=2))
            outp = ph3.enter_context(tc.tile_pool(name="outp", bufs=2))

            PTs = [
                pt_pool.tile([128, 8, 512], F8, tag="pt1", name="PT1"),
                pt_pool.tile([128, 16, 512], F8, tag="pt2", name="PT2"),
            ]
            Pb = pt_pool.tile([128, 2, 128], BF16, tag="pb", name="Pb")

            def st_fused(ps_st):
                # one pass over key tiles; each KT stationary chunk-pair
                # serves BOTH slot groups' S^T matmuls (kt<8)
                for kt in range(16):
                    work = []   # (group, sps, w, col0, f)
                    for g in ((1, 0) if kt < 8 else (1,)):
                        Ls = LIMITS[g * 4:(g + 1) * 4]
                        f = sum(1 for L in Ls if L <= kt)
                        w = (4 - f) * 128
                        col0 = f * 128
                        sps = ps_st.tile([128, 512], F32, tag="st",
                                         name=f"s{g}_{kt}")
                        work.append((g, sps, w, col0, f))
                    for c2 in range(4):
                        for g, sps, w, col0, f in work:
                            nc.tensor.matmul(
                                sps[:, 0:w],
                                KT8[:, 2 * c2:2 * c2 + 2, kt * 128:(kt + 1) * 128],
                                QT8[:, 2 * c2:2 * c2 + 2,
                                    g * 512 + col0: g * 512 + col0 + w],
                                start=(c2 == 0), stop=(c2 == 3),
                                perf_mode=DR,
                            )
                    for g, sps, w, col0, f in work:
                        Ls = LIMITS[g * 4:(g + 1) * 4]
                        if kt == Ls[f] - 2:
                            nc.vector.tensor_add(
                                sps[:, 0:128], sps[:, 0:128],
                                mask_sb[:, g * 256: g * 256 + 128],
                            )
                        elif kt == Ls[f] - 1:
                            nc.vector.tensor_add(
                                sps[:, 0:128], sps[:, 0:128],
                                mask_sb[:, g * 256 + 128: g * 256 + 256],
                            )
                        nc.scalar.activation(
                            PTs[g][:, kt, col0:col0 + w], sps[:, 0:w],
                            mybir.ActivationFunctionType.Exp,
                            bias=0.0, scale=EXP_SCALE,
                        )

            def st_fix(ps_st):
                # bf16 S^T for the fixup slot (g0 col0, true key tiles 0,1)
                for kt in range(2):
                    spb = ps_st.tile([128, 512], F32, tag="st", name=f"sf{kt}")
                    for c in range(8):
                        nc.tensor.matmul(
                            spb[:, 0:128], KTb[:, c, kt * 128:(kt + 1) * 128],
                            QTb[:, c, :],
                            start=(c == 0), stop=(c == 7),
                        )
                    nc.vector.tensor_add(
                        spb[:, 0:128], spb[:, 0:128],
                        mask_sb[:, kt * 128:(kt + 1) * 128],
                    )
                    nc.scalar.activation(
                        Pb[:, kt, :], spb[:, 0:128],
                        mybir.ActivationFunctionType.Exp,
                        bias=0.0, scale=EXP_SCALE,
                    )

            with ExitStack() as st_scope:
                ps_st = st_scope.enter_context(
                    tc.tile_pool(name="ps_st", bufs=3, space="PSUM"))
                st_fused(ps_st)

            ps_o = ph3.enter_context(tc.tile_pool(name="ps_o", bufs=3, space="PSUM"))
            ps_rs = ph3.enter_context(tc.tile_pool(name="ps_rs", bufs=1, space="PSUM"))
            ps_fx = ph3.enter_context(tc.tile_pool(name="ps_fx", bufs=1, space="PSUM"))

            def av_epilogue(slot, O_ps, rs_ps):
                stats = sc_pool.tile([128, 8], F32, tag="stats", name=f"st{slot}")
                recip = stats[:, 0:1]
                nc.vector.reciprocal(recip, rs_ps)
                out_sb = outp.tile([128, D], BF16, tag="osb", name=f"ou{slot}")
                nc.vector.tensor_scalar_mul(out_sb, O_ps, recip)
                eng = nc.scalar if slot % 2 == 0 else nc.gpsimd
                eng.dma_start(out=out_q[slot][:, :], in_=out_sb)

            def av_slot(g, j):
                # fp8 DoubleRow over key-tile pairs; rowsum reuses stationary
                PT = PTs[g]
                slot = g * 4 + j
                L = LIMITS[slot]
                col = j * 128
                O_ps = ps_o.tile([128, D], F32, tag="O", name=f"O{slot}")
                rs_ps = ps_rs.tile([128, 1], F32, tag="rs", name=f"r{slot}")
                L2 = L // 2
                for t2 in range(L2):
                    pt_blk = PT[:, 2 * t2:2 * t2 + 2, col:col + 128]
                    for h in range(2):
                        nc.tensor.matmul(
                            O_ps[:, h * 512:(h + 1) * 512], pt_blk,
                            V8[:, 2 * t2:2 * t2 + 2, h * 512:(h + 1) * 512],
                            start=(t2 == 0), stop=(t2 == L2 - 1),
                            perf_mode=DR,
                        )
                    nc.tensor.matmul(
                        rs_ps, pt_blk, ones8[:, :, 0:1],
                        start=(t2 == 0), stop=(t2 == L2 - 1),
                        perf_mode=DR,
                    )
                av_epilogue(slot, O_ps, rs_ps)

            def av_fix():
                # bf16 AV for the fixup slot (slot 0, L=2)
                O_ps = ps_o.tile([128, D], F32, tag="O", name="Ofix")
                rs_ps = ps_rs.tile([128, 1], F32, tag="rs", name="rfix")
                for kt in range(2):
                    pb_blk = Pb[:, kt, :]
                    for h in range(2):
                        nc.tensor.matmul(
                            O_ps[:, h * 512:(h + 1) * 512], pb_blk,
                            Vb[:, kt, h * 512:(h + 1) * 512],
                            start=(kt == 0), stop=(kt == 1),
                        )
                    nc.tensor.matmul(
                        rs_ps, pb_blk, onesb[:, 0:1],
                        start=(kt == 0), stop=(kt == 1),
                    )
                av_epilogue(0, O_ps, rs_ps)

            # interleave big(g1)/small(g0) slots in descending L; fixup slot
            # (L=2, bf16) last so the end-of-kernel chain is shortest.
            # st_fix sits after the first big slot so the small bf16 gather
            # (gb) has until then to land.
            av_slot(1, 3)
            st_fix(ps_fx)
            av_slot(0, 3)
            for j in (2, 1):
                av_slot(1, j)
                av_slot(0, j)
            av_slot(1, 0)
            av_fix()

    nc.compile()
    return nc


def _masks():
    k = np.arange(128)[:, None]
    q = np.arange(128)[None, :]
    tril_t = np.where(k <= q, 0.0, NEG).astype(np.float32)  # S^T diag block
    fullneg = np.full((128, 128), NEG, np.float32)
    zeros = np.zeros((128, 128), np.float32)
    m_s0 = np.concatenate([tril_t, fullneg, zeros, tril_t], axis=1)
    m_s1 = np.concatenate([zeros, tril_t, tril_t, fullneg], axis=1)
    return m_s0, m_s1


def kernel(x, Wq, Wk, Wv):
    global LAST_EXEC_NS
    x = np.asarray(x, dtype=np.float32)
    Wq = np.asarray(Wq, dtype=np.float32)
    Wk = np.asarray(Wk, dtype=np.float32)
    Wv = np.asarray(Wv, dtype=np.float32)

    if "nc" not in _NC_CACHE:
        _NC_CACHE["nc"] = _build_nc()
    nc = _NC_CACHE["nc"]

    # host pre-transpose: x[b] (N, D) -> (tile, p=d%128, dchunk, token)
    xt_f32 = np.ascontiguousarray(
        x.reshape(B, N_KTILES, 128, 8, 128).transpose(0, 1, 4, 3, 2)
    )  # [B, tile, p, c, q] f32
    xt_all = xt_f32.astype(BF)
    x8_all = xt_f32.astype(F8NP)

    # weights -> [p=d%128, dchunk, ecol]; premuls folded in
    wq_r = np.ascontiguousarray(
        (QK_PREMUL * Wq).reshape(8, 128, 1024).transpose(1, 0, 2).astype(BF))
    wk_r = np.ascontiguousarray(
        (QK_PREMUL * Wk).reshape(8, 128, 1024).transpose(1, 0, 2).astype(BF))
    wv_scaled = np.ascontiguousarray(
        (V_PREMUL * Wv).reshape(8, 128, 1024).transpose(1, 0, 2))
    wv8_r = wv_scaled.astype(F8NP)
    wvb_r = wv_scaled.astype(BF)

    m_s0, m_s1 = _masks()
    in_maps = []
    for c in range(N_CORES):
        b, s = divmod(c, 2)
        in_maps.append({
            "x_kt": np.ascontiguousarray(xt_all[b, s * 8:(s + 1) * 8]),
            "x_kt8": np.ascontiguousarray(x8_all[b, s * 8:(s + 1) * 8]),
            "x_qt": np.ascontiguousarray(xt_all[b, QSETS[s]]),
            "wq": wq_r, "wk": wk_r, "wv8": wv8_r, "wvb": wvb_r,
            "mask": m_s1 if s else m_s0,
        })

    res = run_bass_kernel_spmd(nc, in_maps, list(range(N_CORES)), trace=TRACE)
    LAST_EXEC_NS = res.exec_time_ns

    out = np.empty((B, N, D), dtype=np.float32)
    for c in range(N_CORES):
        b, s = divmod(c, 2)
        oq = np.asarray(res.results[c]["out_q"], dtype=np.float32)
        for j, g in enumerate(QSETS[s]):
            out[b, g * 128:(g + 1) * 128, :] = oq[j]
    return out


# revision 15
# speedup vs baseline: 1.5880x; 1.0107x over previous
"""Causal attention (B=4, N=2048, D=1024) on 8 Trainium2 NeuronCores.

v4 design (vs v3 fp8, 183us):
  * Intra-pair K/V projection split: the two cores of a batch pair each
    project only THEIR 8 key tiles of K^T (bf16) and V (fp8 DoubleRow),
    then exchange halves with pair AllGather collectives
    (replica_groups [[0,1],[2,3],[4,5],[6,7]]) staged through DRAM
    bounce buffers on the gpsimd ring.  AllGather output index h is
    replica h's data, so the gathered tensor is in TRUE key order on
    both cores -- the program stays SPMD-uniform.  Saves 27.3us (K) +
    14.5us (V) of duplicated PE work per core; the exchange hides under
    the Q projection.
  * fp8(e4m3) DoubleRow matmuls for V proj, S^T and AV; Q/K stay bf16.
    Scales folded into host weights: wq,wk = 4*W; wv8 = e4m3(32*Wv);
    exp scale = (1/32)/16; rowsum ones = 32.0 cancels the V premul.
  * Early-row fixup: each core's L=2 slot (q-tile 0 for s=0, 1 for s=1)
    runs a full bf16 path (bf16 K^T/Q^T/P/V for true keys 0..255).  The
    bf16 KTb/Vb are computed from s=0's local tiles and broadcast via a
    small bf16 AllGather (s=1's contribution is ignored).  numpy-sim of
    the exact mix: max rel err 4.4e-3 (tolerance 2e-2).
  * All input DMAs on ONE logical queue (sync/HWDGE) in priority order:
    SDMA engines round-robin active queues at packet granularity, so a
    single queue is the only way to get true priority + full ~340 GB/s.
  * Core 2b+s handles batch b; s=0 takes query tiles {0,2,4,6, 9,11,13,15},
    s=1 takes {1,3,5,7, 8,10,12,14} -- both 68 causal key-tile pairs.
"""
import sys

sys.path.insert(0, "/opt/trn_rl_repo")

from contextlib import ExitStack

import numpy as np
import ml_dtypes

import concourse.bass as bass
import concourse.mybir as mybir
import concourse.tile as tile
from concourse import bacc
from concourse.bass_utils import run_bass_kernel_spmd

B, N, D = 4, 2048, 1024
N_CORES = 8
N_SLOTS = 8
N_KTILES = 16
SCALE = 1.0 / 32.0   # 1/sqrt(D)
QK_PREMUL = 4.0      # folded into wq/wk on host
V_PREMUL = 32.0      # folded into wv on host
EXP_SCALE = SCALE / (QK_PREMUL * QK_PREMUL)
NEG = -1.0e9

F32 = mybir.dt.float32
BF16 = mybir.dt.bfloat16
F8 = mybir.dt.float8e4
DR = mybir.MatmulPerfMode.DoubleRow
BF = ml_dtypes.bfloat16
F8NP = ml_dtypes.float8_e4m3

PAIRS = [[0, 1], [2, 3], [4, 5], [6, 7]]

# query-tile sets per parity slot s (ascending); both have sum(g+1) == 68
QSETS = [
    [0, 2, 4, 6, 9, 11, 13, 15],
    [1, 3, 5, 7, 8, 10, 12, 14],
]
# uniform program limits per slot (key tiles 0..L-1 computed)
LIMITS = [2, 4, 6, 8, 10, 12, 14, 16]

_NC_CACHE = {}
TRACE = False
LAST_EXEC_NS = None


def _build_nc():
    nc = bacc.Bacc(None, target_bir_lowering=False, debug=False, num_devices=8)

    # x tile layouts: [tile, p=d%128, dchunk, token]; _kt = own key half
    x_kt = nc.declare_dram_parameter("x_kt", [8, 128, 8, 128], BF16, isOutput=False)
    x_kt8 = nc.declare_dram_parameter("x_kt8", [8, 128, 8, 128], F8, isOutput=False)
    x_qt = nc.declare_dram_parameter("x_qt", [N_SLOTS, 128, 8, 128], BF16, isOutput=False)
    # weights: wq/wv [p=d%128, dchunk, ecol]; wk e-block-major so the K
    # projection's first e-block needs only 0.25 MB of weight DMA
    wq = nc.declare_dram_parameter("wq", [128, 8, 1024], BF16, isOutput=False)
    wk = nc.declare_dram_parameter("wk", [8, 128, 8, 128], BF16, isOutput=False)
    wv8 = nc.declare_dram_parameter("wv8", [128, 8, 1024], F8, isOutput=False)
    wvb = nc.declare_dram_parameter("wvb", [128, 8, 1024], BF16, isOutput=False)
    mask_in = nc.declare_dram_parameter("mask", [128, 512], F32, isOutput=False)
    out_q = nc.declare_dram_parameter("out_q", [N_SLOTS, 128, D], BF16, isOutput=True)

    with tile.TileContext(nc) as tc, ExitStack() as top:
        consts = top.enter_context(tc.tile_pool(name="consts", bufs=1))
        kt_pool = top.enter_context(tc.tile_pool(name="ktp", bufs=1))
        v_pool = top.enter_context(tc.tile_pool(name="vp", bufs=1))
        qt_pool = top.enter_context(tc.tile_pool(name="qtp", bufs=1))
        dram = top.enter_context(tc.tile_pool(name="dram", bufs=6, space="DRAM"))

        ones8 = consts.tile([128, 2, 16], F8)
        nc.vector.memset(ones8, V_PREMUL)
        onesb = consts.tile([128, 8], BF16)
        nc.vector.memset(onesb, V_PREMUL)
        mask_sb = consts.tile([128, 512], F32)

        KT8 = kt_pool.tile([128, 8, N], F8)        # [p=e%128, echunk, key]
        KTb = kt_pool.tile([128, 8, 256], BF16)    # bf16 keys 0..255 (fixup)
        QT8 = qt_pool.tile([128, 8, 1024], F8)     # [p=e%128, echunk, qcol]
        QTb = qt_pool.tile([128, 8, 128], BF16)    # bf16 fixup q-tile (g0 col 0)
        V8 = v_pool.tile([128, N_KTILES, D], F8)   # [p=key%128, ktile, e]
        Vb = v_pool.tile([128, 2, D], BF16)        # bf16 V kt0/1 (fixup)

        # DRAM bounce buffers for the pair exchanges
        stV = dram.tile([128, 8, 1024], F8)        # own V half
        gV = dram.tile([2, 128, 8, 1024], F8)
        stK = dram.tile([128, 8, 1024], F8)        # own K^T half
        gK = dram.tile([2, 128, 8, 1024], F8)
        stb = dram.tile([128, 4, 1024], BF16)      # KTb (0:2) + Vb (2:4)
        gb = dram.tile([2, 128, 4, 1024], BF16)
        st0 = dram.tile([128, 16], F8)             # warmup collective bounce
        g0 = dram.tile([2, 128, 16], F8)

        # tiny warmup AllGather issued first: absorbs the ~12us first-
        # collective mesh sync so the K gather runs at warm latency
        nc.gpsimd.dma_start(out=st0[:], in_=ones8[:, 0, :])
        nc.gpsimd.collective_compute(
            "AllGather", mybir.AluOpType.bypass, replica_groups=PAIRS,
            ins=[st0.opt()], outs=[g0.opt()])

        with ExitStack() as ph12:
            x8_pool = ph12.enter_context(tc.tile_pool(name="x8p", bufs=1))
            xt_pool = ph12.enter_context(tc.tile_pool(name="xtp", bufs=1))
            qxt_pool = ph12.enter_context(tc.tile_pool(name="qxt", bufs=1))
            w_pool = ph12.enter_context(tc.tile_pool(name="wp", bufs=1))
            hf_pool = ph12.enter_context(tc.tile_pool(name="hf", bufs=1))
            ps_mm = ph12.enter_context(tc.tile_pool(name="ps_mm", bufs=8, space="PSUM"))

            # ---- input DMAs: ONE queue (sync/HWDGE), strict priority ----
            wv8_sb = w_pool.tile([128, 8, 1024], F8, tag="wv8")
            x8 = x8_pool.tile([128, 8, 8, 128], F8, tag="x8")
            wk_sb = w_pool.tile([128, 8, 8, 128], BF16, tag="wk")  # [p,eblk,c,ecol]
            xT = xt_pool.tile([128, 8, 8, 128], BF16, tag="xT")
            wvb_sb = w_pool.tile([128, 8, 1024], BF16, tag="wvb")
            QXT = qxt_pool.tile([128, 8, 8, 128], BF16, tag="qx")
            wq_sb = w_pool.tile([128, 8, 1024], BF16, tag="wq")

            nc.sync.dma_start(
                out=wk_sb[:, 0:1], in_=wk[0:1].rearrange("e p c q -> p e c q"))
            nc.sync.dma_start(
                out=xT[:, 0:4], in_=x_kt[0:4].rearrange("t p c q -> p t c q"))
            nc.sync.dma_start(
                out=wk_sb[:, 1:8], in_=wk[1:8].rearrange("e p c q -> p e c q"))
            nc.sync.dma_start(
                out=xT[:, 4:8], in_=x_kt[4:8].rearrange("t p c q -> p t c q"))
            nc.sync.dma_start(out=wv8_sb, in_=wv8[:, :, :])
            nc.sync.dma_start(
                out=x8, in_=x_kt8[:].rearrange("t p c q -> p t c q"))
            nc.sync.dma_start(out=wvb_sb, in_=wvb[:, :, :])
            nc.sync.dma_start(
                out=QXT, in_=x_qt[:].rearrange("s p c q -> p s c q"))
            nc.sync.dma_start(out=wq_sb, in_=wq[:, :, :])
            nc.sync.dma_start(out=mask_sb, in_=mask_in[:, :])

            vhalf = hf_pool.tile([128, 8, 1024], F8, tag="vh")
            khalf = hf_pool.tile([128, 8, 1024], F8, tag="kh")
            ktb_sb = hf_pool.tile([128, 8, 256], BF16, tag="ktb")
            vb_sb = hf_pool.tile([128, 2, 1024], BF16, tag="vbs")

            def v_half():
                # fp8 DoubleRow; stationary x chunk-pair shared by both e-halves
                for lt in range(8):
                    vps = [ps_mm.tile([128, 512], F32, tag="mm", name=f"v{lt}_{eh}")
                           for eh in range(2)]
                    for c2 in range(4):
                        for eh in range(2):
                            nc.tensor.matmul(
                                vps[eh],
                                x8[:, lt, 2 * c2:2 * c2 + 2, :],
                                wv8_sb[:, 2 * c2:2 * c2 + 2, eh * 512:(eh + 1) * 512],
                                start=(c2 == 0), stop=(c2 == 3),
                                perf_mode=DR,
                            )
                    for eh in range(2):
                        nc.vector.tensor_copy(
                            vhalf[:, lt, eh * 512:(eh + 1) * 512], vps[eh])

            def k_pass(tg):
                # bf16 K^T projection for one 4-tile group of the own half
                for e in range(8):
                    kps = ps_mm.tile([128, 512], F32, tag="mm", name=f"k{tg}_{e}")
                    for c in range(8):
                        nc.tensor.matmul(
                            kps, wk_sb[:, e, c, :],
                            xT[:, tg * 4:(tg + 1) * 4, c, :],
                            start=(c == 0), stop=(c == 7),
                        )
                    nc.vector.tensor_copy(khalf[:, e, tg * 512:(tg + 1) * 512], kps)
                    if tg == 0:
                        nc.vector.tensor_copy(ktb_sb[:, e, :], kps[:, 0:256])

            def vb_fix():
                # bf16 V for local tiles 0,1 (true kt0/1 on s=0 cores; the
                # gather takes replica 0's data, s=1's result is discarded)
                for t in range(2):
                    vbp = [ps_mm.tile([128, 512], F32, tag="mm", name=f"vb{t}_{eh}")
                           for eh in range(2)]
                    for c in range(8):
                        for eh in range(2):
                            nc.tensor.matmul(
                                vbp[eh], xT[:, t, c, :],
                                wvb_sb[:, c, eh * 512:(eh + 1) * 512],
                                start=(c == 0), stop=(c == 7),
                            )
                    for eh in range(2):
                        nc.vector.tensor_copy(
                            vb_sb[:, t, eh * 512:(eh + 1) * 512], vbp[eh])

            def proj_queries():
                # both slot groups; stationary W chunk shared across groups
                for e in range(8):
                    qps = [ps_mm.tile([128, 512], F32, tag="mm", name=f"q{e}_{g}")
                           for g in range(2)]
                    for c in range(8):
                        for g in range(2):
                            nc.tensor.matmul(
                                qps[g], wq_sb[:, c, e * 128:(e + 1) * 128],
                                QXT[:, g * 4:(g + 1) * 4, c, :],
                                start=(c == 0), stop=(c == 7),
                            )
                    for g in range(2):
                        nc.vector.tensor_copy(QT8[:, e, g * 512:(g + 1) * 512], qps[g])
                    nc.vector.tensor_copy(QTb[:, e, :], qps[0][:, 0:128])

            # --- projections + pair exchange (collectives on gpsimd ring) ---
            # K first: the S^T phase needs the gathered K^T earliest, and the
            # CC core processes collectives strictly in issue order.
            k_pass(0)
            k_pass(1)
            nc.gpsimd.dma_start(out=stK[:], in_=khalf)
            nc.gpsimd.collective_compute(
                "AllGather", mybir.AluOpType.bypass, replica_groups=PAIRS,
                ins=[stK.opt()], outs=[gK.opt()])
            # readbacks (true key order: gather index h = replica h's half)
            for h in range(2):
                nc.gpsimd.dma_start(
                    out=KT8[:, :, h * 1024:(h + 1) * 1024], in_=gK[h][:, :, :])
            v_half()
            nc.gpsimd.dma_start(out=stV[:], in_=vhalf)
            nc.gpsimd.collective_compute(
                "AllGather", mybir.AluOpType.bypass, replica_groups=PAIRS,
                ins=[stV.opt()], outs=[gV.opt()])
            for h in range(2):
                nc.gpsimd.dma_start(
                    out=V8[:, h * 8:(h + 1) * 8, :], in_=gV[h][:, :, :])
            vb_fix()
            nc.gpsimd.dma_start(
                out=stb[:, 0:2, :].rearrange("p a b -> p (a b)"),
                in_=ktb_sb.rearrange("p a b -> p (a b)"))
            nc.gpsimd.dma_start(out=stb[:, 2:4, :], in_=vb_sb)
            nc.gpsimd.collective_compute(
                "AllGather", mybir.AluOpType.bypass, replica_groups=PAIRS,
                ins=[stb.opt()], outs=[gb.opt()])
            nc.gpsimd.dma_start(
                out=KTb.rearrange("p a b -> p (a b)"),
                in_=gb[0][:, 0:2, :].rearrange("p a b -> p (a b)"))
            nc.gpsimd.dma_start(out=Vb, in_=gb[0][:, 2:4, :])
            proj_queries()

        # ---- attention: S^T per key tile, then AV with P^T stationary ----
        with ExitStack() as ph3:
            pt_pool = ph3.enter_context(tc.tile_pool(name="ptp", bufs=1))
            sc_pool = ph3.enter_context(tc.tile_pool(name="scp", bufs=2))
            outp = ph3.enter_context(tc.tile_pool(name="outp", bufs=2))

            PTs = [
                pt_pool.tile([128, 8, 512], F8, tag="pt1", name="PT1"),
                pt_pool.tile([128, 16, 512], F8, tag="pt2", name="PT2"),
            ]
            Pb = pt_pool.tile([128, 2, 128], BF16, tag="pb", name="Pb")

            def st_fused(ps_st):
                # one pass over key tiles; each KT stationary chunk-pair
                # serves BOTH slot groups' S^T matmuls (kt<8)
                for kt in range(16):
                    work = []   # (group, sps, w, col0, f)
                    for g in ((1, 0) if kt < 8 else (1,)):
                        Ls = LIMITS[g * 4:(g + 1) * 4]
                        f = sum(1 for L in Ls if L <= kt)
                        w = (4 - f) * 128
                        col0 = f * 128
                        sps = ps_st.tile([128, 512], F32, tag="st",
                                         name=f"s{g}_{kt}")
                        work.append((g, sps, w, col0, f))
                    for c2 in range(4):
                        for g, sps, w, col0, f in work:
                            nc.tensor.matmul(
                                sps[:, 0:w],
                                KT8[:, 2 * c2:2 * c2 + 2, kt * 128:(kt + 1) * 128],
                                QT8[:, 2 * c2:2 * c2 + 2,
                                    g * 512 + col0: g * 512 + col0 + w],
                                start=(c2 == 0), stop=(c2 == 3),
                                perf_mode=DR,
                            )
                    for g, sps, w, col0, f in work:
                        Ls = LIMITS[g * 4:(g + 1) * 4]
                        if kt == Ls[f] - 2:
                            nc.vector.tensor_add(
                                sps[:, 0:128], sps[:, 0:128],
                                mask_sb[:, g * 256: g * 256 + 128],
                            )
                        elif kt == Ls[f] - 1:
                            nc.vector.tensor_add(
                                sps[:, 0:128], sps[:, 0:128],
                                mask_sb[:, g * 256 + 128: g * 256 + 256],
                            )
                        nc.scalar.activation(
                            PTs[g][:, kt, col0:col0 + w], sps[:, 0:w],
                            mybir.ActivationFunctionType.Exp,
                            bias=0.0, scale=EXP_SCALE,
                        )

            def st_fix(ps_st):
                # bf16 S^T for the fixup slot (g0 col0, true key tiles 0,1)
                for kt in range(2):
                    spb = ps_st.tile([128, 512], F32, tag="st", name=f"sf{kt}")
                    for c in range(8):
                        nc.tensor.matmul(
                            spb[:, 0:128], KTb[:, c, kt * 128:(kt + 1) * 128],
                            QTb[:, c, :],
                            start=(c == 0), stop=(c == 7),
                        )
                    nc.vector.tensor_add(
                        spb[:, 0:128], spb[:, 0:128],
                        mask_sb[:, kt * 128:(kt + 1) * 128],
                    )
                    nc.scalar.activation(
                        Pb[:, kt, :], spb[:, 0:128],
                        mybir.ActivationFunctionType.Exp,
                        bias=0.0, scale=EXP_SCALE,
                    )

            with ExitStack() as st_scope:
                ps_st = st_scope.enter_context(
                    tc.tile_pool(name="ps_st", bufs=3, space="PSUM"))
                st_fused(ps_st)

            ps_o = ph3.enter_context(tc.tile_pool(name="ps_o", bufs=3, space="PSUM"))
            ps_rs = ph3.enter_context(tc.tile_pool(name="ps_rs", bufs=1, space="PSUM"))
            ps_fx = ph3.enter_context(tc.tile_pool(name="ps_fx", bufs=1, space="PSUM"))

            def av_epilogue(slot, O_ps, rs_ps):
                stats = sc_pool.tile([128, 8], F32, tag="stats", name=f"st{slot}")
                recip = stats[:, 0:1]
                nc.vector.reciprocal(recip, rs_ps)
                out_sb = outp.tile([128, D], BF16, tag="osb", name=f"ou{slot}")
                nc.vector.tensor_scalar_mul(out_sb, O_ps, recip)
                eng = nc.scalar if slot % 2 == 0 else nc.gpsimd
                eng.dma_start(out=out_q[slot][:, :], in_=out_sb)

            def av_slot(g, j):
                # fp8 DoubleRow over key-tile pairs; rowsum reuses stationary
                PT = PTs[g]
                slot = g * 4 + j
                L = LIMITS[slot]
                col = j * 128
                O_ps = ps_o.tile([128, D], F32, tag="O", name=f"O{slot}")
                rs_ps = ps_rs.tile([128, 1], F32, tag="rs", name=f"r{slot}")
                L2 = L // 2
                for t2 in range(L2):
                    pt_blk = PT[:, 2 * t2:2 * t2 + 2, col:col + 128]
                    for h in range(2):
                        nc.tensor.matmul(
                            O_ps[:, h * 512:(h + 1) * 512], pt_blk,
                            V8[:, 2 * t2:2 * t2 + 2, h * 512:(h + 1) * 512],
                            start=(t2 == 0), stop=(t2 == L2 - 1),
                            perf_mode=DR,
                        )
                    nc.tensor.matmul(
                        rs_ps, pt_blk, ones8[:, :, 0:1],
                        start=(t2 == 0), stop=(t2 == L2 - 1),
                        perf_mode=DR,
                    )
                av_epilogue(slot, O_ps, rs_ps)

            def av_fix():
                # bf16 AV for the fixup slot (slot 0, L=2)
                O_ps = ps_o.tile([128, D], F32, tag="O", name="Ofix")
                rs_ps = ps_rs.tile([128, 1], F32, tag="rs", name="rfix")
                for kt in range(2):
                    pb_blk = Pb[:, kt, :]
                    for h in range(2):
                        nc.tensor.matmul(
                            O_ps[:, h * 512:(h + 1) * 512], pb_blk,
                            Vb[:, kt, h * 512:(h + 1) * 512],
                            start=(kt == 0), stop=(kt == 1),
                        )
                    nc.tensor.matmul(
                        rs_ps, pb_blk, onesb[:, 0:1],
                        start=(kt == 0), stop=(kt == 1),
                    )
                av_epilogue(0, O_ps, rs_ps)

            # interleave big(g1)/small(g0) slots in descending L; fixup slot
            # (L=2, bf16) last so the end-of-kernel chain is shortest.
            # st_fix sits after the first big slot so the small bf16 gather
            # (gb) has until then to land.
            av_slot(1, 3)
            st_fix(ps_fx)
            av_slot(0, 3)
            for j in (2, 1):
                av_slot(1, j)
                av_slot(0, j)
            av_slot(1, 0)
            av_fix()

    nc.compile()
    return nc


def _masks():
    k = np.arange(128)[:, None]
    q = np.arange(128)[None, :]
    tril_t = np.where(k <= q, 0.0, NEG).astype(np.float32)  # S^T diag block
    fullneg = np.full((128, 128), NEG, np.float32)
    zeros = np.zeros((128, 128), np.float32)
    m_s0 = np.concatenate([tril_t, fullneg, zeros, tril_t], axis=1)
    m_s1 = np.concatenate([zeros, tril_t, tril_t, fullneg], axis=1)
    return m_s0, m_s1


def kernel(x, Wq, Wk, Wv):
    global LAST_EXEC_NS
    x = np.asarray(x, dtype=np.float32)
    Wq = np.asarray(Wq, dtype=np.float32)
    Wk = np.asarray(Wk, dtype=np.float32)
    Wv = np.asarray(Wv, dtype=np.float32)

    if "nc" not in _NC_CACHE:
        _NC_CACHE["nc"] = _build_nc()
    nc = _NC_CACHE["nc"]

    # host pre-transpose: x[b] (N, D) -> (tile, p=d%128, dchunk, token)
    xt_f32 = np.ascontiguousarray(
        x.reshape(B, N_KTILES, 128, 8, 128).transpose(0, 1, 4, 3, 2)
    )  # [B, tile, p, c, q] f32
    xt_all = xt_f32.astype(BF)
    x8_all = xt_f32.astype(F8NP)

    # weights -> [p=d%128, dchunk, ecol]; premuls folded in
    wq_r = np.ascontiguousarray(
        (QK_PREMUL * Wq).reshape(8, 128, 1024).transpose(1, 0, 2).astype(BF))
    wk_r = np.ascontiguousarray(
        (QK_PREMUL * Wk).reshape(8, 128, 8, 128).transpose(2, 1, 0, 3).astype(BF))
    wv_scaled = np.ascontiguousarray(
        (V_PREMUL * Wv).reshape(8, 128, 1024).transpose(1, 0, 2))
    wv8_r = wv_scaled.astype(F8NP)
    wvb_r = wv_scaled.astype(BF)

    m_s0, m_s1 = _masks()
    in_maps = []
    for c in range(N_CORES):
        b, s = divmod(c, 2)
        in_maps.append({
            "x_kt": np.ascontiguousarray(xt_all[b, s * 8:(s + 1) * 8]),
            "x_kt8": np.ascontiguousarray(x8_all[b, s * 8:(s + 1) * 8]),
            "x_qt": np.ascontiguousarray(xt_all[b, QSETS[s]]),
            "wq": wq_r, "wk": wk_r, "wv8": wv8_r, "wvb": wvb_r,
            "mask": m_s1 if s else m_s0,
        })

    res = run_bass_kernel_spmd(nc, in_maps, list(range(N_CORES)), trace=TRACE)
    LAST_EXEC_NS = res.exec_time_ns

    out = np.empty((B, N, D), dtype=np.float32)
    for c in range(N_CORES):
        b, s = divmod(c, 2)
        oq = np.asarray(res.results[c]["out_q"], dtype=np.float32)
        for j, g in enumerate(QSETS[s]):
            out[b, g * 128:(g + 1) * 128, :] = oq[j]
    return out
